# revision 24
# baseline (speedup 1.0000x reference)
"""GQA kernel for Trainium2, 8-way tensor-parallel over kv heads.

Problem (hardcoded): B=1, S=2048, D_MODEL=2048, HQ=32, HKV=8, DK=64, causal.
Sharding: core h owns kv head h and its 4 query heads. Weights are sliced,
transposed and cast to bf16 on host; x is replicated (transposed bf16). Each
core computes a partial y (its heads projected through its slice of wo); host
sums the 8 partials and adds bo.

On-chip dataflow per core (all matmuls bf16 with fp32 PSUM accumulation):
  xT, wqkvT, woT arrive pre-transposed -> no PE transposes for x/weights
  qkvT[f, s] projection, evacuated via DVE with fused per-partition bias
  scores: heads processed in pairs (g_even at array rows 0-63, g_odd at rows
    64-127 via duplicated K) -> the two 64-contraction matmuls run
    CONCURRENTLY in separate PE row groups (tile_position auto-derived)
  expT = exp(0.125 * scoresT) on ACT, causal strips only; diagonal blocks
    masked by gpsimd affine_select on the leading 128 columns only
  AV: col-tiled pair -- head g_even -> PSUM partitions 0-63, g_odd -> 64-127
    of one bank (concurrent), V is 64-wide; denominators via ones-column
    matmuls into partitions 0 / 32 of a second bank (concurrent pair)
  normalize: reciprocal_approx_fast + tiny broadcast matmuls + one DVE mul
  y = outT.T @ woT per 128-row block, interleaved into attention slack

Emission is software-pipelined with a filler queue: F(ti-1) and QKV(ti+1)
matmul chunks are popped between attention strips so PE stays busy while ACT
computes exp (ACT paces the attention phase).
"""

from collections import deque

import numpy as np

S = 2048
D = 2048
HQ, HKV, GRP, DK = 32, 8, 4, 64
QF = GRP * DK            # 256 query features per core
NF = QF + 2 * DK         # 384 projected features per core
N_CORES = 8
NT = S // 128            # 16 d-tiles
NI = S // 512            # 4 i-tiles

_cache = {}


def _build(debug_dumps=False):
    import concourse.bass as bass
    import concourse.mybir as mybir
    from concourse import bacc, tile
    from concourse.masks import make_identity
    from contextlib import ExitStack

    f32 = mybir.dt.float32
    bf16 = mybir.dt.bfloat16
    Exp = mybir.ActivationFunctionType.Exp

    nc = bacc.Bacc(
        "TRN2",
        target_bir_lowering=False,
        debug=False,
        enable_asserts=False,
        num_devices=N_CORES,
    )
    # host layouts (see _in_maps): xt[ni*128+p, dt*512+s'] = x[ni*512+s', dt*128+p]
    xt_d = nc.dram_tensor("xt", [NI * 128, NT * 512], bf16, kind="ExternalInput").ap()
    wt_d = nc.dram_tensor("wt", [128, NT * NF], bf16, kind="ExternalInput").ap()
    wo_d = nc.dram_tensor("wob", [128, 2 * D], bf16, kind="ExternalInput").ap()
    bq_d = nc.dram_tensor("bq", [NF, 1], f32, kind="ExternalInput").ap()
    y_d = nc.dram_tensor("y", [S, D], bf16, kind="ExternalOutput").ap()
    dbg = {}
    if debug_dumps:
        for nm, shp in [("d_qkvT0", [128, S]), ("d_qkvT1", [128, S]),
                        ("d_qkvT2", [128, S]), ("d_kdup", [128, S]),
                        ("d_vS", [128, NT * DK]), ("d_outT0", [128, S]),
                        ("d_outT1", [128, S])]:
            dbg[nm] = nc.dram_tensor(nm, shp, bf16, kind="ExternalOutput").ap()
        for nm, shp in [("d_aug", [128, 512]), ("d_dn", [33, 512]),
                        ("d_bcS", [128, 512])]:
            dbg[nm] = nc.dram_tensor(nm, shp, f32, kind="ExternalOutput").ap()

    with tile.TileContext(nc) as tc, ExitStack() as ctx:
        const = ctx.enter_context(tc.tile_pool(name="const", bufs=1))
        expp = ctx.enter_context(tc.tile_pool(name="expp", bufs=6))
        ysbp = ctx.enter_context(tc.tile_pool(name="ysbp", bufs=6))
        rcpp = ctx.enter_context(tc.tile_pool(name="rcpp", bufs=3))
        # PSUM (8 banks): sc 2x[128,1024]=4, av 1, dn/bc 1, qkv 1, f 1
        ps_sc = ctx.enter_context(tc.tile_pool(name="ps_sc", bufs=2, space="PSUM"))
        ps_av = ctx.enter_context(tc.tile_pool(name="ps_av", bufs=1, space="PSUM"))
        ps_dn = ctx.enter_context(tc.tile_pool(name="ps_dn", bufs=1, space="PSUM"))
        ps_qkv = ctx.enter_context(tc.tile_pool(name="ps_qkv", bufs=1, space="PSUM"))
        ps_f = ctx.enter_context(tc.tile_pool(name="ps_f", bufs=1, space="PSUM"))

        # ---- constants ----
        ident_bf = const.tile([128, 128], bf16)
        make_identity(nc, ident_bf)
        onescol = const.tile([128, 1], bf16)
        nc.gpsimd.memset(onescol, 1.0)
        onesB = const.tile([33, 64], bf16)
        nc.gpsimd.memset(onesB, 1.0)

        # ---- persistent SBUF ----
        XT = const.tile([128, NI * NT * 512], bf16)
        XTv = XT.rearrange("p (n t s) -> p n t s", n=NI, t=NT)
        wT = const.tile([128, NT * NF], bf16)
        wTv = wT.rearrange("p (t f) -> p t f", t=NT)
        woT = const.tile([128, 2 * D], bf16)
        woTv = woT.rearrange("p (t m) -> p t m", t=2)
        # qkvT[0] = Q heads g0|g1 (rows 0-63|64-127), [1] = g2|g3, [2] = K|V
        qkvT = [const.tile([128, S], bf16, name=f"qkvT{i}") for i in range(3)]
        kdup = const.tile([128, S], bf16)           # K duplicated at rows 64-127
        vS = const.tile([128, NT * DK], bf16)       # V as [s%128, strip, f]
        vSv = vS.rearrange("p (j f) -> p j f", j=NT)
        outT = [const.tile([128, S], bf16, name=f"outT{i}") for i in range(2)]
        btl = [const.tile([128, 1], f32, name=f"btl{i}") for i in range(3)]

        # ---- upfront DMAs (x slabs ordered by first use; the ni=0 slab and
        # wT arrive in dt-quad chunks so QKV(0)'s first matmuls start early) ----
        wTq = wT.rearrange("p (q r) -> p q r", q=4)
        for q in range(4):
            nc.sync.dma_start(out=XTv[:, 0, 4 * q:4 * q + 4, :],
                              in_=xt_d[0:128, q * 2048:(q + 1) * 2048])
            nc.sync.dma_start(out=wTq[:, q],
                              in_=wt_d[:, q * 4 * NF:(q + 1) * 4 * NF])
        for mi in range(3):
            nc.sync.dma_start(out=btl[mi], in_=bq_d[mi * 128:(mi + 1) * 128, :])
        nc.sync.dma_start(out=XTv[:, 1], in_=xt_d[128:256, :])
        nc.sync.dma_start(out=woT, in_=wo_d)
        nc.sync.dma_start(out=XTv[:, 2], in_=xt_d[256:384, :])
        nc.sync.dma_start(out=XTv[:, 3], in_=xt_d[384:512, :])

        # ---- stage emitters ----
        psq_live = {}

        def emit_qkv_chunk(nis, mi, c, pool=None, tg="ps_qkv"):
            # 4 of the 16 contraction tiles of the [128f, 512s] projection;
            # paired nis share the stationary weights (walrus dedupes the
            # LDWEIGHTS of consecutive same-lhsT matmuls)
            if c == 0:
                for ni in nis:
                    psq_live[(ni, mi)] = (pool or ps_qkv).tile(
                        [128, 512], f32, tag=tg, name="psq")
            for dt in range(4 * c, 4 * c + 4):
                for ni in nis:
                    nc.tensor.matmul(
                        psq_live[(ni, mi)],
                        lhsT=wTv[:, dt, mi * 128:(mi + 1) * 128],
                        rhs=XTv[:, ni, dt, :],
                        start=(dt == 0),
                        stop=(dt == NT - 1),
                    )
            if c == 3:
                for ni in nis:
                    psq = psq_live.pop((ni, mi))
                    nc.vector.tensor_scalar_add(
                        qkvT[mi][:, ni * 512:(ni + 1) * 512], psq, btl[mi])
                    if mi == 2:
                        nc.sync.dma_start(
                            out=kdup[64:128, ni * 512:(ni + 1) * 512],
                            in_=qkvT[2][0:DK, ni * 512:(ni + 1) * 512],
                        )

        def emit_vt(ni):
            # V strips of this i-tile transposed into vS via PE
            psv = ps_f.tile([128, 4 * DK], bf16, tag="ps_f", name="psv")
            for j in range(4):
                nc.tensor.transpose(
                    psv[:, j * DK:(j + 1) * DK],
                    qkvT[2][64:128, (4 * ni + j) * 128:(4 * ni + j + 1) * 128],
                    ident_bf[64:128, 64:128],
                )
            nc.vector.tensor_copy(
                vSv[:, 4 * ni:4 * ni + 4, :],
                psv.rearrange("p (a b) -> p a b", a=4),
            )

        def emit_f(sb, mi2):
            psy = ps_f.tile([128, 512], f32, tag="ps_f", name="psy")
            for ft in range(2):
                nc.tensor.matmul(
                    psy,
                    lhsT=outT[ft][:, sb * 128:(sb + 1) * 128],
                    rhs=woTv[:, ft, mi2 * 512:(mi2 + 1) * 512],
                    start=(ft == 0),
                    stop=(ft == 1),
                )
            ysb = ysbp.tile([128, 512], bf16, tag="ysbp", name="ysb")
            if (sb + mi2) % 2 == 0:
                nc.vector.tensor_copy(ysb, psy)
            else:
                nc.scalar.copy(ysb, psy)
            nc.sync.dma_start(
                out=y_d[sb * 128:(sb + 1) * 128, mi2 * 512:(mi2 + 1) * 512],
                in_=ysb,
            )

        def emit_scores(ti, p, bj):
            # head pair p: g_even at rows 0-63 (K source qkvT[2]), g_odd at
            # rows 64-127 (kdup). The two matmuls auto-derive tile_position
            # (0,0)/(64,0) -> they run concurrently in separate PE row groups.
            # Strips are computed full-width; causally dead columns are zeroed
            # after exp (keeps every PSUM byte initialized for the single exp).
            k = bj - 4 * ti
            off = 128 * k if k > 0 else 0
            W = 512 - off
            sc = ps_sc.tile([128, 1024], f32, tag="ps_sc", name="sc")
            # E half stored i-aligned at [off:512], O half packed at [512:512+W]
            # so the exp range [off:512+W] is contiguous and fully written
            nc.tensor.matmul(
                sc[:, off:512],
                lhsT=qkvT[2][0:DK, bj * 128:(bj + 1) * 128],
                rhs=qkvT[p][0:DK, ti * 512 + off:(ti + 1) * 512],
                start=True, stop=True,
            )
            nc.tensor.matmul(
                sc[:, 512:512 + W],
                lhsT=kdup[64:128, bj * 128:(bj + 1) * 128],
                rhs=qkvT[p][64:128, ti * 512 + off:(ti + 1) * 512],
                start=True, stop=True,
            )
            expT = expp.tile([128, 1024], bf16, tag="expp", name="expT")
            nc.scalar.activation(expT[:, off:512 + W], sc[:, off:512 + W],
                                 Exp, scale=0.125)
            if k >= 0:
                # zero j > i in the leading 128-col diagonal block of each half
                for lo in (off, 512):
                    nc.gpsimd.affine_select(
                        out=expT[:, lo:lo + 128],
                        in_=expT[:, lo:lo + 128],
                        compare_op=mybir.AluOpType.is_ge,
                        fill=0.0, base=0,
                        pattern=[[1, 128]], channel_multiplier=-1,
                    )
            return expT, off, W

        def emit_avdn(bj, nstr, expT, off, W, aug, dn):
            first, last = bj == 0, bj == nstr - 1
            # col-tiled AV pair: g_even -> psum partitions 0-63, g_odd -> 64-127
            nc.tensor.matmul(
                aug[0:DK, off:512], lhsT=vSv[:, bj, :], rhs=expT[:, off:512],
                start=first, stop=last, skip_group_check=True,
            )
            nc.tensor.matmul(
                aug[DK:128, off:512], lhsT=vSv[:, bj, :],
                rhs=expT[:, 512:512 + W],
                start=first, stop=last, skip_group_check=True,
            )
            # denominators: ones-column matmuls -> partitions 0 / 32 (concurrent)
            nc.tensor.matmul(
                dn[0:1, off:512], lhsT=onescol, rhs=expT[:, off:512],
                start=first, stop=last, skip_group_check=True,
            )
            nc.tensor.matmul(
                dn[32:33, off:512], lhsT=onescol, rhs=expT[:, 512:512 + W],
                start=first, stop=last, skip_group_check=True,
            )

        def emit_norm(ti, p, aug, dn):
            if debug_dumps and ti == 3 and p == 1:
                daug = const.tile([128, 512], f32, name="daug")
                nc.vector.tensor_copy(daug, aug)
                nc.sync.dma_start(out=dbg["d_aug"], in_=daug)
                ddn = const.tile([33, 512], f32, name="ddn")
                nc.vector.tensor_copy(ddn[0:1, :], dn[0:1, :])
                nc.vector.tensor_copy(ddn[32:33, :], dn[32:33, :])
                nc.sync.dma_start(out=dbg["d_dn"], in_=ddn)
            rcp = rcpp.tile([33, 512], f32, tag="rcp", name="rcp")
            rcpB = rcpp.tile([33, 512], bf16, tag="rcpB", name="rcpB")
            # full-tile op: reciprocal_approx_fast mis-executes on HW for APs
            # with base partition != 0 (probed), so cover rows 0..32 in one op
            # (rows 1-31 are junk-in/junk-out, initialized once below)
            nc.vector.reciprocal_approx_fast(rcp, dn[0:33, :])
            nc.vector.tensor_copy(rcpB, rcp)
            # broadcast recips over the pair's rows, reusing dn's bank
            nc.tensor.matmul(dn[0:DK, :], lhsT=onesB[0:1, :], rhs=rcpB[0:1, :],
                             start=True, stop=True, skip_group_check=True)
            nc.tensor.matmul(dn[DK:128, :], lhsT=onesB[32:33, :],
                             rhs=rcpB[32:33, :], start=True, stop=True,
                             skip_group_check=True)
            # DVE may read only one PSUM operand: stage bc in SBUF via ACT
            bcS = rcpp.tile([128, 512], bf16, tag="bcS", name="bcS")
            nc.scalar.copy(bcS, dn)
            if debug_dumps and ti == 3 and p == 1:
                dbcS = const.tile([128, 512], f32, name="dbcS")
                nc.vector.tensor_copy(dbcS, bcS)
                nc.sync.dma_start(out=dbg["d_bcS"], in_=dbcS)
            nc.vector.tensor_mul(outT[p][:, ti * 512:(ti + 1) * 512], aug, bcS)

        # ---- pipelined schedule ----
        fill = deque()

        def pump(n):
            for _ in range(n):
                if not fill:
                    return
                fill.popleft()()

        # startup QKV(0) rotates through the (still idle) score banks so the
        # three mi-groups never stall on a single bank's evacuation
        for mi in range(3):
            for c in range(4):
                emit_qkv_chunk((0,), mi, c, pool=ps_sc, tg="ps_sc")
        emit_vt(0)

        for ti in range(NI):
            if ti < NI - 1:
                for mi in range(3):
                    for c in range(4):
                        fill.append(lambda ni=ti + 1, mi=mi, c=c:
                                    emit_qkv_chunk((ni,), mi, c))
                fill.append(lambda ni=ti + 1: emit_vt(ni))
            if ti >= 1:
                for sb in range(4 * (ti - 1), 4 * ti):
                    for mi2 in range(4):
                        fill.append(lambda sb=sb, mi2=mi2: emit_f(sb, mi2))
            nstr = 4 * ti + 4
            for p in range(2):
                aug = ps_av.tile([128, 512], f32, tag="ps_av", name="aug")
                dn = ps_dn.tile([128, 512], f32, tag="ps_dn", name="dn")
                # keep rows 1-31 finite/nonzero and owned by this tile for the
                # full-tile reciprocal (row 0 is overwritten by the start=True
                # denominator matmul)
                nc.vector.memset(dn[0:32, :], 1.0)
                prev = None
                for bj in range(nstr):
                    cur = emit_scores(ti, p, bj)
                    pump(1)
                    if prev is not None:
                        emit_avdn(bj - 1, nstr, *prev, aug, dn)
                    if len(fill) > 8:
                        pump(1)
                    prev = cur
                pump(1)
                emit_avdn(nstr - 1, nstr, *prev, aug, dn)
                emit_norm(ti, p, aug, dn)
            pump(len(fill))
        # tail: attention banks are idle now -- rotate the final F through the
        # freed ps_sc/av/dn banks with alternating evac engines so the PE
        # stays dense (and HAM stays warm) to the end
        tail_pools = [(ps_sc, "ps_sc"), (ps_av, "ps_av"),
                      (ps_sc, "ps_sc"), (ps_dn, "ps_dn")]
        for i, (sb, mi2) in enumerate(
                (sb, mi2) for sb in range(4 * (NI - 1), 4 * NI)
                for mi2 in range(4)):
            pool, tg = tail_pools[i % 4]
            psy = pool.tile([128, 512], f32, tag=tg, name="psyt")
            for ft in range(2):
                nc.tensor.matmul(
                    psy,
                    lhsT=outT[ft][:, sb * 128:(sb + 1) * 128],
                    rhs=woTv[:, ft, mi2 * 512:(mi2 + 1) * 512],
                    start=(ft == 0),
                    stop=(ft == 1),
                )
            ysb = ysbp.tile([128, 512], bf16, tag="ysbp", name="ysb")
            if i % 2 == 0:
                nc.vector.tensor_copy(ysb, psy)
            else:
                nc.scalar.copy(ysb, psy)
            nc.sync.dma_start(
                out=y_d[sb * 128:(sb + 1) * 128, mi2 * 512:(mi2 + 1) * 512],
                in_=ysb,
            )

        if debug_dumps:
            for nm, src in [("d_qkvT0", qkvT[0]), ("d_qkvT1", qkvT[1]),
                            ("d_qkvT2", qkvT[2]), ("d_kdup", kdup),
                            ("d_vS", vS), ("d_outT0", outT[0]),
                            ("d_outT1", outT[1])]:
                nc.sync.dma_start(out=dbg[nm], in_=src)

    nc.compile()
    return nc


def _get_nc():
    if "nc" not in _cache:
        _cache["nc"] = _build()
    return _cache["nc"]


def _in_maps(x, wq, bq, wk, bk, wv, bv, wo):
    import ml_dtypes

    bf = ml_dtypes.bfloat16
    x = np.asarray(x, np.float32)
    # xt[ni*128+p, dt*512+s'] = x[ni*512+s', dt*128+p]  (shared by all cores)
    xT = np.ascontiguousarray(x.T).astype(bf)                    # [d, s]
    xt = np.ascontiguousarray(
        xT.reshape(NT, 128, NI, 512).transpose(2, 1, 0, 3)
    ).reshape(NI * 128, NT * 512)
    maps = []
    for h in range(N_CORES):
        qs = slice(h * QF, (h + 1) * QF)
        ks = slice(h * DK, (h + 1) * DK)
        wqkv = np.concatenate([wq[qs], wk[ks], wv[ks]], axis=0)  # [384, 2048]
        wqkvT = np.ascontiguousarray(wqkv.T.astype(np.float32)).astype(bf)
        wt = np.ascontiguousarray(
            wqkvT.reshape(NT, 128, NF).transpose(1, 0, 2)
        ).reshape(128, NT * NF)
        woT = np.ascontiguousarray(wo[:, qs].T.astype(np.float32)).astype(bf)
        wob = np.ascontiguousarray(
            woT.reshape(2, 128, D).transpose(1, 0, 2)
        ).reshape(128, 2 * D)
        bqkv = np.concatenate([bq[qs], bk[ks], bv[ks]], axis=0).astype(np.float32)
        maps.append({
            "xt": xt,
            "wt": wt,
            "wob": wob,
            "bq": np.ascontiguousarray(bqkv[:, None]),
        })
    return maps


def _run(inputs, trace=False, tmpdir=None):
    from concourse.bass_utils import run_bass_kernel_spmd

    nc = _get_nc()
    x = np.asarray(inputs["x"])[0]
    maps = _in_maps(
        x,
        np.asarray(inputs["wq"]), np.asarray(inputs["bq"]),
        np.asarray(inputs["wk"]), np.asarray(inputs["bk"]),
        np.asarray(inputs["wv"]), np.asarray(inputs["bv"]),
        np.asarray(inputs["wo"]),
    )
    res = run_bass_kernel_spmd(
        nc, maps, list(range(N_CORES)), trace=trace, tmpdir=tmpdir
    )
    y = np.zeros((S, D), dtype=np.float32)
    for i in range(N_CORES):
        y += np.asarray(res.results[i]["y"]).astype(np.float32)
    y += np.asarray(inputs["bo"])[None, :]
    return y[None], res


def kernel(**inputs):
    y, _ = _run(inputs, trace=False)
    return y.astype(np.float32)


# revision 28
# speedup vs baseline: 1.0396x; 1.0396x over previous
"""GQA kernel for Trainium2, 8-way tensor-parallel over kv heads.

Problem (hardcoded): B=1, S=2048, D_MODEL=2048, HQ=32, HKV=8, DK=64, causal.
Sharding: core h owns kv head h and its 4 query heads. Weights are sliced,
transposed and cast to bf16 on host; x is replicated (transposed bf16). Each
core computes a partial y (its heads projected through its slice of wo); host
sums the 8 partials and adds bo.

On-chip dataflow per core (all matmuls bf16 with fp32 PSUM accumulation):
  xT, wqkvT, woT arrive pre-transposed -> no PE transposes for x/weights
  qkvT[f, s] projection, evacuated via DVE with fused per-partition bias
  scores: heads processed in pairs (g_even at array rows 0-63, g_odd at rows
    64-127 via duplicated K) -> the two 64-contraction matmuls run
    CONCURRENTLY in separate PE row groups (tile_position auto-derived)
  expT = exp(0.125 * scoresT) on ACT, causal strips only; diagonal blocks
    masked by gpsimd affine_select on the leading 128 columns only
  AV: col-tiled pair -- head g_even -> PSUM partitions 0-63, g_odd -> 64-127
    of one bank (concurrent), V is 64-wide; denominators via ones-column
    matmuls into partitions 0 / 32 of a second bank (concurrent pair)
  normalize: reciprocal_approx_fast + tiny broadcast matmuls + one DVE mul
  y = outT.T @ woT per 128-row block, interleaved into attention slack

Emission is software-pipelined with a filler queue: F(ti-1) and QKV(ti+1)
matmul chunks are popped between attention strips so PE stays busy while ACT
computes exp (ACT paces the attention phase).
"""

from collections import deque

import numpy as np

S = 2048
D = 2048
HQ, HKV, GRP, DK = 32, 8, 4, 64
QF = GRP * DK            # 256 query features per core
NF = QF + 2 * DK         # 384 projected features per core
N_CORES = 8
NT = S // 128            # 16 d-tiles
NI = S // 512            # 4 i-tiles

_cache = {}


def _build(debug_dumps=False):
    import concourse.bass as bass
    import concourse.mybir as mybir
    from concourse import bacc, tile
    from concourse.masks import make_identity
    from contextlib import ExitStack

    f32 = mybir.dt.float32
    bf16 = mybir.dt.bfloat16
    Exp = mybir.ActivationFunctionType.Exp

    nc = bacc.Bacc(
        "TRN2",
        target_bir_lowering=False,
        debug=False,
        enable_asserts=False,
        num_devices=N_CORES,
    )
    # host layouts (see _in_maps): xt[ni*128+p, dt*512+s'] = x[ni*512+s', dt*128+p]
    xt_d = nc.dram_tensor("xt", [NI * 128, NT * 512], bf16, kind="ExternalInput").ap()
    wt_d = nc.dram_tensor("wt", [128, NT * NF], bf16, kind="ExternalInput").ap()
    wo_d = nc.dram_tensor("wob", [128, 2 * D], bf16, kind="ExternalInput").ap()
    bq_d = nc.dram_tensor("bq", [NF, 1], f32, kind="ExternalInput").ap()
    y_d = nc.dram_tensor("y", [S, D], bf16, kind="ExternalOutput").ap()
    dbg = {}
    if debug_dumps:
        for nm, shp in [("d_qkvT0", [128, S]), ("d_qkvT1", [128, S]),
                        ("d_qkvT2", [128, S]), ("d_kdup", [128, S]),
                        ("d_vS", [128, NT * DK]), ("d_outT0", [128, S]),
                        ("d_outT1", [128, S])]:
            dbg[nm] = nc.dram_tensor(nm, shp, bf16, kind="ExternalOutput").ap()
        for nm, shp in [("d_aug", [128, 512]), ("d_dn", [33, 512]),
                        ("d_bcS", [128, 512])]:
            dbg[nm] = nc.dram_tensor(nm, shp, f32, kind="ExternalOutput").ap()

    with tile.TileContext(nc) as tc, ExitStack() as ctx:
        const = ctx.enter_context(tc.tile_pool(name="const", bufs=1))
        expp = ctx.enter_context(tc.tile_pool(name="expp", bufs=4))
        ysbp = ctx.enter_context(tc.tile_pool(name="ysbp", bufs=4))
        rcpp = ctx.enter_context(tc.tile_pool(name="rcpp", bufs=2))
        # PSUM (8 banks): sc 2x[128,1024]=4, av 1, dn/bc 1, qkv 1, f 1
        ps_sc = ctx.enter_context(tc.tile_pool(name="ps_sc", bufs=2, space="PSUM"))
        ps_av = ctx.enter_context(tc.tile_pool(name="ps_av", bufs=1, space="PSUM"))
        ps_dn = ctx.enter_context(tc.tile_pool(name="ps_dn", bufs=1, space="PSUM"))
        ps_qkv = ctx.enter_context(tc.tile_pool(name="ps_qkv", bufs=1, space="PSUM"))
        ps_f = ctx.enter_context(tc.tile_pool(name="ps_f", bufs=1, space="PSUM"))

        # ---- constants ----
        ident_bf = const.tile([128, 128], bf16)
        make_identity(nc, ident_bf)
        onescol = const.tile([128, 1], bf16)
        nc.gpsimd.memset(onescol, 1.0)
        onesB = const.tile([33, 64], bf16)
        nc.gpsimd.memset(onesB, 1.0)

        # ---- persistent SBUF ----
        XT = const.tile([128, NI * NT * 512], bf16)
        XTv = XT.rearrange("p (n t s) -> p n t s", n=NI, t=NT)
        wT = const.tile([128, NT * NF], bf16)
        wTv = wT.rearrange("p (t f) -> p t f", t=NT)
        woT = const.tile([128, 2 * D], bf16)
        woTv = woT.rearrange("p (t m) -> p t m", t=2)
        # qkvT[0] = Q heads g0|g1 (rows 0-63|64-127), [1] = g2|g3, [2] = K|V
        qkvT = [const.tile([128, S], bf16, name=f"qkvT{i}") for i in range(3)]
        kdup = const.tile([128, S], bf16)           # K duplicated at rows 64-127
        vS = const.tile([128, NT * DK], bf16)       # V as [s%128, strip, f]
        vSv = vS.rearrange("p (j f) -> p j f", j=NT)
        outT = [const.tile([128, S], bf16, name=f"outT{i}") for i in range(2)]
        btl = [const.tile([128, 1], f32, name=f"btl{i}") for i in range(3)]

        # ---- upfront DMAs (x slabs ordered by first use; the ni=0 slab and
        # wT arrive in dt-quad chunks so QKV(0)'s first matmuls start early) ----
        wTq = wT.rearrange("p (q r) -> p q r", q=4)
        for q in range(4):
            nc.sync.dma_start(out=XTv[:, 0, 4 * q:4 * q + 4, :],
                              in_=xt_d[0:128, q * 2048:(q + 1) * 2048])
            nc.sync.dma_start(out=wTq[:, q],
                              in_=wt_d[:, q * 4 * NF:(q + 1) * 4 * NF])
        for mi in range(3):
            nc.sync.dma_start(out=btl[mi], in_=bq_d[mi * 128:(mi + 1) * 128, :])
        nc.sync.dma_start(out=XTv[:, 1], in_=xt_d[128:256, :])
        nc.sync.dma_start(out=woT, in_=wo_d)
        nc.sync.dma_start(out=XTv[:, 2], in_=xt_d[256:384, :])
        nc.sync.dma_start(out=XTv[:, 3], in_=xt_d[384:512, :])

        # ---- stage emitters ----
        psq_live = {}

        def emit_qkv_chunk(nis, mi, c, pool=None, tg="ps_qkv"):
            # 4 of the 16 contraction tiles of the [128f, 512s] projection;
            # paired nis share the stationary weights (walrus dedupes the
            # LDWEIGHTS of consecutive same-lhsT matmuls)
            if c == 0:
                for ni in nis:
                    psq_live[(ni, mi)] = (pool or ps_qkv).tile(
                        [128, 512], f32, tag=tg, name="psq")
            for dt in range(4 * c, 4 * c + 4):
                for ni in nis:
                    nc.tensor.matmul(
                        psq_live[(ni, mi)],
                        lhsT=wTv[:, dt, mi * 128:(mi + 1) * 128],
                        rhs=XTv[:, ni, dt, :],
                        start=(dt == 0),
                        stop=(dt == NT - 1),
                    )
            if c == 3:
                for ni in nis:
                    psq = psq_live.pop((ni, mi))
                    nc.vector.tensor_scalar_add(
                        qkvT[mi][:, ni * 512:(ni + 1) * 512], psq, btl[mi])
                    if mi == 2:
                        nc.sync.dma_start(
                            out=kdup[64:128, ni * 512:(ni + 1) * 512],
                            in_=qkvT[2][0:DK, ni * 512:(ni + 1) * 512],
                        )

        def emit_vt(ni):
            # V strips of this i-tile transposed into vS via PE
            psv = ps_f.tile([128, 4 * DK], bf16, tag="ps_f", name="psv")
            for j in range(4):
                nc.tensor.transpose(
                    psv[:, j * DK:(j + 1) * DK],
                    qkvT[2][64:128, (4 * ni + j) * 128:(4 * ni + j + 1) * 128],
                    ident_bf[64:128, 64:128],
                )
            nc.vector.tensor_copy(
                vSv[:, 4 * ni:4 * ni + 4, :],
                psv.rearrange("p (a b) -> p a b", a=4),
            )

        def emit_f(sb, mi2):
            psy = ps_f.tile([128, 512], f32, tag="ps_f", name="psy")
            for ft in range(2):
                nc.tensor.matmul(
                    psy,
                    lhsT=outT[ft][:, sb * 128:(sb + 1) * 128],
                    rhs=woTv[:, ft, mi2 * 512:(mi2 + 1) * 512],
                    start=(ft == 0),
                    stop=(ft == 1),
                )
            ysb = ysbp.tile([128, 512], bf16, tag="ysbp", name="ysb")
            nc.vector.tensor_copy(ysb, psy)
            nc.sync.dma_start(
                out=y_d[sb * 128:(sb + 1) * 128, mi2 * 512:(mi2 + 1) * 512],
                in_=ysb,
            )

        def emit_scores(ti, p, bj):
            # head pair p: g_even at rows 0-63 (K source qkvT[2]), g_odd at
            # rows 64-127 (kdup). The two matmuls auto-derive tile_position
            # (0,0)/(64,0) -> they run concurrently in separate PE row groups.
            # Strips are computed full-width; causally dead columns are zeroed
            # after exp (keeps every PSUM byte initialized for the single exp).
            k = bj - 4 * ti
            off = 128 * k if k > 0 else 0
            W = 512 - off
            sc = ps_sc.tile([128, 1024], f32, tag="ps_sc", name="sc")
            # E half stored i-aligned at [off:512], O half packed at [512:512+W]
            # so the exp range [off:512+W] is contiguous and fully written
            nc.tensor.matmul(
                sc[:, off:512],
                lhsT=qkvT[2][0:DK, bj * 128:(bj + 1) * 128],
                rhs=qkvT[p][0:DK, ti * 512 + off:(ti + 1) * 512],
                start=True, stop=True,
            )
            nc.tensor.matmul(
                sc[:, 512:512 + W],
                lhsT=kdup[64:128, bj * 128:(bj + 1) * 128],
                rhs=qkvT[p][64:128, ti * 512 + off:(ti + 1) * 512],
                start=True, stop=True,
            )
            expT = expp.tile([128, 1024], bf16, tag="expp", name="expT")
            nc.scalar.activation(expT[:, off:512 + W], sc[:, off:512 + W],
                                 Exp, scale=0.125)
            if k >= 0:
                # zero j > i in the leading 128-col diagonal block of each half
                for lo in (off, 512):
                    nc.gpsimd.affine_select(
                        out=expT[:, lo:lo + 128],
                        in_=expT[:, lo:lo + 128],
                        compare_op=mybir.AluOpType.is_ge,
                        fill=0.0, base=0,
                        pattern=[[1, 128]], channel_multiplier=-1,
                    )
            return expT, off, W

        def emit_avdn(bj, expT, off, W, aug, dn, first, last):
            # col-tiled AV pair: g_even -> psum partitions 0-63, g_odd -> 64-127
            nc.tensor.matmul(
                aug[0:DK, off:512], lhsT=vSv[:, bj, :], rhs=expT[:, off:512],
                start=first, stop=last, skip_group_check=True,
            )
            nc.tensor.matmul(
                aug[DK:128, off:512], lhsT=vSv[:, bj, :],
                rhs=expT[:, 512:512 + W],
                start=first, stop=last, skip_group_check=True,
            )
            # denominators: ones-column matmuls -> partitions 0 / 32 (concurrent)
            nc.tensor.matmul(
                dn[0:1, off:512], lhsT=onescol, rhs=expT[:, off:512],
                start=first, stop=last, skip_group_check=True,
            )
            nc.tensor.matmul(
                dn[32:33, off:512], lhsT=onescol, rhs=expT[:, 512:512 + W],
                start=first, stop=last, skip_group_check=True,
            )

        def emit_norm(ti, p, aug, dn):
            if debug_dumps and ti == 3 and p == 1:
                daug = const.tile([128, 512], f32, name="daug")
                nc.vector.tensor_copy(daug, aug)
                nc.sync.dma_start(out=dbg["d_aug"], in_=daug)
                ddn = const.tile([33, 512], f32, name="ddn")
                nc.vector.tensor_copy(ddn[0:1, :], dn[0:1, :])
                nc.vector.tensor_copy(ddn[32:33, :], dn[32:33, :])
                nc.sync.dma_start(out=dbg["d_dn"], in_=ddn)
            rcp = rcpp.tile([33, 512], f32, tag="rcp", name="rcp")
            rcpB = rcpp.tile([33, 512], bf16, tag="rcpB", name="rcpB")
            # full-tile op: reciprocal_approx_fast mis-executes on HW for APs
            # with base partition != 0 (probed), so cover rows 0..32 in one op
            # (rows 1-31 are junk-in/junk-out, initialized once below)
            nc.vector.reciprocal_approx_fast(rcp, dn[0:33, :])
            nc.vector.tensor_copy(rcpB, rcp)
            # broadcast recips over the pair's rows, reusing dn's bank
            nc.tensor.matmul(dn[0:DK, :], lhsT=onesB[0:1, :], rhs=rcpB[0:1, :],
                             start=True, stop=True, skip_group_check=True)
            nc.tensor.matmul(dn[DK:128, :], lhsT=onesB[32:33, :],
                             rhs=rcpB[32:33, :], start=True, stop=True,
                             skip_group_check=True)
            # DVE may read only one PSUM operand: stage bc in SBUF via ACT
            bcS = rcpp.tile([128, 512], bf16, tag="bcS", name="bcS")
            nc.scalar.copy(bcS, dn)
            if debug_dumps and ti == 3 and p == 1:
                dbcS = const.tile([128, 512], f32, name="dbcS")
                nc.vector.tensor_copy(dbcS, bcS)
                nc.sync.dma_start(out=dbg["d_bcS"], in_=dbcS)
            nc.vector.tensor_mul(outT[p][:, ti * 512:(ti + 1) * 512], aug, bcS)

        # ---- pipelined schedule ----
        fill = deque()

        def pump(n):
            for _ in range(n):
                if not fill:
                    return
                fill.popleft()()

        # warm the PE clock (HAM) with identity matmuls on resident SBUF
        # data while the input DMAs stream in
        warm = ps_qkv.tile([128, 512], f32, tag="ps_qkv", name="warm")
        for _ in range(28):
            nc.tensor.matmul(warm[:, 0:128], lhsT=ident_bf, rhs=ident_bf,
                             start=True, stop=True)
        # startup QKV(0) rotates through the (still idle) score banks so the
        # three mi-groups never stall on a single bank's evacuation
        for mi in range(3):
            for c in range(4):
                emit_qkv_chunk((0,), mi, c, pool=ps_sc, tg="ps_sc")
        emit_vt(0)

        for ti in range(NI):
            if ti < NI - 1:
                for mi in range(3):
                    for c in range(4):
                        fill.append(lambda ni=ti + 1, mi=mi, c=c:
                                    emit_qkv_chunk((ni,), mi, c))
                fill.append(lambda ni=ti + 1: emit_vt(ni))
            if ti >= 1:
                for sb in range(4 * (ti - 1), 4 * ti):
                    for mi2 in range(4):
                        fill.append(lambda sb=sb, mi2=mi2: emit_f(sb, mi2))
            nstr = 4 * ti + 4
            for p in range(2):
                aug = ps_av.tile([128, 512], f32, tag="ps_av", name="aug")
                dn = ps_dn.tile([128, 512], f32, tag="ps_dn", name="dn")
                # keep rows 1-31 finite/nonzero and owned by this tile for the
                # full-tile reciprocal (row 0 is overwritten by the start=True
                # denominator matmul)
                nc.vector.memset(dn[0:32, :], 1.0)
                # full-width strip 0 first (uniform start=True write), then
                # diagonal strips early so their exp->mask->AV latency hides
                # under later strips' scores instead of the pair's tail
                order = [0] + list(range(nstr - 1, 0, -1))
                prev = None
                for idx, bj in enumerate(order):
                    cur = (bj, emit_scores(ti, p, bj))
                    pump(1)
                    if prev is not None:
                        emit_avdn(prev[0], *prev[1], aug, dn,
                                  first=(idx == 1), last=False)
                    prev = cur
                pump(1)
                emit_avdn(prev[0], *prev[1], aug, dn, first=False, last=True)
                emit_norm(ti, p, aug, dn)
            pump(len(fill))
        # tail: attention banks are idle now -- rotate the final F through the
        # freed ps_sc/av/dn banks with alternating evac engines so the PE
        # stays dense (and HAM stays warm) to the end
        tail_pools = [(ps_sc, "ps_sc"), (ps_av, "ps_av"),
                      (ps_sc, "ps_sc"), (ps_dn, "ps_dn")]
        for i, (sb, mi2) in enumerate(
                (sb, mi2) for sb in range(4 * (NI - 1), 4 * NI)
                for mi2 in range(4)):
            pool, tg = tail_pools[i % 4]
            psy = pool.tile([128, 512], f32, tag=tg, name="psyt")
            for ft in range(2):
                nc.tensor.matmul(
                    psy,
                    lhsT=outT[ft][:, sb * 128:(sb + 1) * 128],
                    rhs=woTv[:, ft, mi2 * 512:(mi2 + 1) * 512],
                    start=(ft == 0),
                    stop=(ft == 1),
                )
            ysb = ysbp.tile([128, 512], bf16, tag="ysbp", name="ysb")
            if i % 2 == 0:
                nc.vector.tensor_copy(ysb, psy)
            else:
                nc.scalar.copy(ysb, psy)
            nc.sync.dma_start(
                out=y_d[sb * 128:(sb + 1) * 128, mi2 * 512:(mi2 + 1) * 512],
                in_=ysb,
            )

        if debug_dumps:
            for nm, src in [("d_qkvT0", qkvT[0]), ("d_qkvT1", qkvT[1]),
                            ("d_qkvT2", qkvT[2]), ("d_kdup", kdup),
                            ("d_vS", vS), ("d_outT0", outT[0]),
                            ("d_outT1", outT[1])]:
                nc.sync.dma_start(out=dbg[nm], in_=src)

    nc.compile()
    return nc


def _get_nc():
    if "nc" not in _cache:
        _cache["nc"] = _build()
    return _cache["nc"]


def _in_maps(x, wq, bq, wk, bk, wv, bv, wo):
    import ml_dtypes

    bf = ml_dtypes.bfloat16
    x = np.asarray(x, np.float32)
    # xt[ni*128+p, dt*512+s'] = x[ni*512+s', dt*128+p]  (shared by all cores)
    xT = np.ascontiguousarray(x.T).astype(bf)                    # [d, s]
    xt = np.ascontiguousarray(
        xT.reshape(NT, 128, NI, 512).transpose(2, 1, 0, 3)
    ).reshape(NI * 128, NT * 512)
    maps = []
    for h in range(N_CORES):
        qs = slice(h * QF, (h + 1) * QF)
        ks = slice(h * DK, (h + 1) * DK)
        wqkv = np.concatenate([wq[qs], wk[ks], wv[ks]], axis=0)  # [384, 2048]
        wqkvT = np.ascontiguousarray(wqkv.T.astype(np.float32)).astype(bf)
        wt = np.ascontiguousarray(
            wqkvT.reshape(NT, 128, NF).transpose(1, 0, 2)
        ).reshape(128, NT * NF)
        woT = np.ascontiguousarray(wo[:, qs].T.astype(np.float32)).astype(bf)
        wob = np.ascontiguousarray(
            woT.reshape(2, 128, D).transpose(1, 0, 2)
        ).reshape(128, 2 * D)
        bqkv = np.concatenate([bq[qs], bk[ks], bv[ks]], axis=0).astype(np.float32)
        maps.append({
            "xt": xt,
            "wt": wt,
            "wob": wob,
            "bq": np.ascontiguousarray(bqkv[:, None]),
        })
    return maps


def _run(inputs, trace=False, tmpdir=None):
    from concourse.bass_utils import run_bass_kernel_spmd

    nc = _get_nc()
    x = np.asarray(inputs["x"])[0]
    maps = _in_maps(
        x,
        np.asarray(inputs["wq"]), np.asarray(inputs["bq"]),
        np.asarray(inputs["wk"]), np.asarray(inputs["bk"]),
        np.asarray(inputs["wv"]), np.asarray(inputs["bv"]),
        np.asarray(inputs["wo"]),
    )
    res = run_bass_kernel_spmd(
        nc, maps, list(range(N_CORES)), trace=trace, tmpdir=tmpdir
    )
    y = np.zeros((S, D), dtype=np.float32)
    for i in range(N_CORES):
        y += np.asarray(res.results[i]["y"]).astype(np.float32)
    y += np.asarray(inputs["bo"])[None, :]
    return y[None], res


def kernel(**inputs):
    y, _ = _run(inputs, trace=False)
    return y.astype(np.float32)


# revision 29
# speedup vs baseline: 1.0540x; 1.0139x over previous
"""GQA kernel for Trainium2, 8-way tensor-parallel over kv heads.

Problem (hardcoded): B=1, S=2048, D_MODEL=2048, HQ=32, HKV=8, DK=64, causal.
Sharding: core h owns kv head h and its 4 query heads. Weights are sliced,
transposed and cast to bf16 on host; x is replicated (transposed bf16). Each
core computes a partial y (its heads projected through its slice of wo); host
sums the 8 partials and adds bo.

On-chip dataflow per core (all matmuls bf16 with fp32 PSUM accumulation):
  xT, wqkvT, woT arrive pre-transposed -> no PE transposes for x/weights
  qkvT[f, s] projection, evacuated via DVE with fused per-partition bias
  scores: heads processed in pairs (g_even at array rows 0-63, g_odd at rows
    64-127 via duplicated K) -> the two 64-contraction matmuls run
    CONCURRENTLY in separate PE row groups (tile_position auto-derived)
  expT = exp(0.125 * scoresT) on ACT, causal strips only; diagonal blocks
    masked by gpsimd affine_select on the leading 128 columns only
  AV: col-tiled pair -- head g_even -> PSUM partitions 0-63, g_odd -> 64-127
    of one bank (concurrent), V is 64-wide; denominators via ones-column
    matmuls into partitions 0 / 32 of a second bank (concurrent pair)
  normalize: reciprocal_approx_fast + tiny broadcast matmuls + one DVE mul
  y = outT.T @ woT per 128-row block, interleaved into attention slack

Emission is software-pipelined with a filler queue: F(ti-1) and QKV(ti+1)
matmul chunks are popped between attention strips so PE stays busy while ACT
computes exp (ACT paces the attention phase).
"""

from collections import deque

import numpy as np

S = 2048
D = 2048
HQ, HKV, GRP, DK = 32, 8, 4, 64
QF = GRP * DK            # 256 query features per core
NF = QF + 2 * DK         # 384 projected features per core
N_CORES = 8
NT = S // 128            # 16 d-tiles
NI = S // 512            # 4 i-tiles

_cache = {}


def _build(debug_dumps=False):
    import concourse.bass as bass
    import concourse.mybir as mybir
    from concourse import bacc, tile
    from concourse.masks import make_identity
    from contextlib import ExitStack

    f32 = mybir.dt.float32
    bf16 = mybir.dt.bfloat16
    Exp = mybir.ActivationFunctionType.Exp

    nc = bacc.Bacc(
        "TRN2",
        target_bir_lowering=False,
        debug=False,
        enable_asserts=False,
        num_devices=N_CORES,
    )
    # host layouts (see _in_maps): xt[ni*128+p, dt*512+s'] = x[ni*512+s', dt*128+p]
    xt_d = nc.dram_tensor("xt", [NI * 128, NT * 512], bf16, kind="ExternalInput").ap()
    wt_d = nc.dram_tensor("wt", [128, NT * NF], bf16, kind="ExternalInput").ap()
    wo_d = nc.dram_tensor("wob", [128, 2 * D], bf16, kind="ExternalInput").ap()
    bq_d = nc.dram_tensor("bq", [NF, 1], f32, kind="ExternalInput").ap()
    y_d = nc.dram_tensor("y", [S, D], bf16, kind="ExternalOutput").ap()
    dbg = {}
    if debug_dumps:
        for nm, shp in [("d_qkvT0", [128, S]), ("d_qkvT1", [128, S]),
                        ("d_qkvT2", [128, S]), ("d_kdup", [128, S]),
                        ("d_vS", [128, NT * DK]), ("d_outT0", [128, S]),
                        ("d_outT1", [128, S])]:
            dbg[nm] = nc.dram_tensor(nm, shp, bf16, kind="ExternalOutput").ap()
        for nm, shp in [("d_aug", [128, 512]), ("d_dn", [33, 512]),
                        ("d_bcS", [128, 512])]:
            dbg[nm] = nc.dram_tensor(nm, shp, f32, kind="ExternalOutput").ap()

    with tile.TileContext(nc) as tc, ExitStack() as ctx:
        const = ctx.enter_context(tc.tile_pool(name="const", bufs=1))
        expp = ctx.enter_context(tc.tile_pool(name="expp", bufs=4))
        ysbp = ctx.enter_context(tc.tile_pool(name="ysbp", bufs=4))
        rcpp = ctx.enter_context(tc.tile_pool(name="rcpp", bufs=2))
        # PSUM (8 banks): sc 2x[128,1024]=4, av 1, dn/bc 1, qkv 1, f 1
        ps_sc = ctx.enter_context(tc.tile_pool(name="ps_sc", bufs=2, space="PSUM"))
        ps_av = ctx.enter_context(tc.tile_pool(name="ps_av", bufs=1, space="PSUM"))
        ps_dn = ctx.enter_context(tc.tile_pool(name="ps_dn", bufs=1, space="PSUM"))
        ps_qkv = ctx.enter_context(tc.tile_pool(name="ps_qkv", bufs=1, space="PSUM"))
        ps_f = ctx.enter_context(tc.tile_pool(name="ps_f", bufs=1, space="PSUM"))

        # ---- constants ----
        ident_bf = const.tile([128, 128], bf16)
        make_identity(nc, ident_bf)
        onescol = const.tile([128, 1], bf16)
        nc.gpsimd.memset(onescol, 1.0)
        onesB = const.tile([33, 64], bf16)
        nc.gpsimd.memset(onesB, 1.0)

        # ---- persistent SBUF ----
        XT = const.tile([128, NI * NT * 512], bf16)
        XTv = XT.rearrange("p (n t s) -> p n t s", n=NI, t=NT)
        wT = const.tile([128, NT * NF], bf16)
        wTv = wT.rearrange("p (t f) -> p t f", t=NT)
        woT = const.tile([128, 2 * D], bf16)
        woTv = woT.rearrange("p (t m) -> p t m", t=2)
        # qkvT[0] = Q heads g0|g1 (rows 0-63|64-127), [1] = g2|g3, [2] = K|V
        qkvT = [const.tile([128, S], bf16, name=f"qkvT{i}") for i in range(3)]
        kdup = const.tile([128, S], bf16)           # K duplicated at rows 64-127
        vS = const.tile([128, NT * DK], bf16)       # V as [s%128, strip, f]
        vSv = vS.rearrange("p (j f) -> p j f", j=NT)
        outT = [const.tile([128, S], bf16, name=f"outT{i}") for i in range(2)]
        btl = [const.tile([128, 1], f32, name=f"btl{i}") for i in range(3)]

        # ---- upfront DMAs (x slabs ordered by first use; the ni=0 slab and
        # wT arrive in dt-quad chunks so QKV(0)'s first matmuls start early) ----
        wTq = wT.rearrange("p (q r) -> p q r", q=4)
        for q in range(4):
            nc.sync.dma_start(out=XTv[:, 0, 4 * q:4 * q + 4, :],
                              in_=xt_d[0:128, q * 2048:(q + 1) * 2048])
            nc.sync.dma_start(out=wTq[:, q],
                              in_=wt_d[:, q * 4 * NF:(q + 1) * 4 * NF])
        for mi in range(3):
            nc.sync.dma_start(out=btl[mi], in_=bq_d[mi * 128:(mi + 1) * 128, :])
        nc.sync.dma_start(out=XTv[:, 1], in_=xt_d[128:256, :])
        nc.sync.dma_start(out=woT, in_=wo_d)
        nc.sync.dma_start(out=XTv[:, 2], in_=xt_d[256:384, :])
        nc.sync.dma_start(out=XTv[:, 3], in_=xt_d[384:512, :])

        # ---- stage emitters ----
        psq_live = {}

        def emit_qkv_chunk(nis, mi, c, pool=None, tg="ps_qkv"):
            # 4 of the 16 contraction tiles of the [128f, 512s] projection;
            # paired nis share the stationary weights (walrus dedupes the
            # LDWEIGHTS of consecutive same-lhsT matmuls)
            if c == 0:
                for ni in nis:
                    psq_live[(ni, mi)] = (pool or ps_qkv).tile(
                        [128, 512], f32, tag=tg, name="psq")
            for dt in range(4 * c, 4 * c + 4):
                for ni in nis:
                    nc.tensor.matmul(
                        psq_live[(ni, mi)],
                        lhsT=wTv[:, dt, mi * 128:(mi + 1) * 128],
                        rhs=XTv[:, ni, dt, :],
                        start=(dt == 0),
                        stop=(dt == NT - 1),
                    )
            if c == 3:
                for ni in nis:
                    psq = psq_live.pop((ni, mi))
                    nc.vector.tensor_scalar_add(
                        qkvT[mi][:, ni * 512:(ni + 1) * 512], psq, btl[mi])
                    if mi == 2:
                        nc.sync.dma_start(
                            out=kdup[64:128, ni * 512:(ni + 1) * 512],
                            in_=qkvT[2][0:DK, ni * 512:(ni + 1) * 512],
                        )

        def emit_vt(ni):
            # V strips of this i-tile transposed into vS via PE
            psv = ps_f.tile([128, 4 * DK], bf16, tag="ps_f", name="psv")
            for j in range(4):
                nc.tensor.transpose(
                    psv[:, j * DK:(j + 1) * DK],
                    qkvT[2][64:128, (4 * ni + j) * 128:(4 * ni + j + 1) * 128],
                    ident_bf[64:128, 64:128],
                )
            nc.vector.tensor_copy(
                vSv[:, 4 * ni:4 * ni + 4, :],
                psv.rearrange("p (a b) -> p a b", a=4),
            )

        def emit_f(sb, mi2):
            psy = ps_f.tile([128, 512], f32, tag="ps_f", name="psy")
            for ft in range(2):
                nc.tensor.matmul(
                    psy,
                    lhsT=outT[ft][:, sb * 128:(sb + 1) * 128],
                    rhs=woTv[:, ft, mi2 * 512:(mi2 + 1) * 512],
                    start=(ft == 0),
                    stop=(ft == 1),
                )
            ysb = ysbp.tile([128, 512], bf16, tag="ysbp", name="ysb")
            nc.vector.tensor_copy(ysb, psy)
            nc.sync.dma_start(
                out=y_d[sb * 128:(sb + 1) * 128, mi2 * 512:(mi2 + 1) * 512],
                in_=ysb,
            )

        def emit_scores(ti, p, bj):
            # head pair p: g_even at rows 0-63 (K source qkvT[2]), g_odd at
            # rows 64-127 (kdup). The two matmuls auto-derive tile_position
            # (0,0)/(64,0) -> they run concurrently in separate PE row groups.
            # Strips are computed full-width; causally dead columns are zeroed
            # after exp (keeps every PSUM byte initialized for the single exp).
            k = bj - 4 * ti
            off = 128 * k if k > 0 else 0
            W = 512 - off
            sc = ps_sc.tile([128, 1024], f32, tag="ps_sc", name="sc")
            # E half stored i-aligned at [off:512], O half packed at [512:512+W]
            # so the exp range [off:512+W] is contiguous and fully written
            nc.tensor.matmul(
                sc[:, off:512],
                lhsT=qkvT[2][0:DK, bj * 128:(bj + 1) * 128],
                rhs=qkvT[p][0:DK, ti * 512 + off:(ti + 1) * 512],
                start=True, stop=True,
            )
            nc.tensor.matmul(
                sc[:, 512:512 + W],
                lhsT=kdup[64:128, bj * 128:(bj + 1) * 128],
                rhs=qkvT[p][64:128, ti * 512 + off:(ti + 1) * 512],
                start=True, stop=True,
            )
            expT = expp.tile([128, 1024], bf16, tag="expp", name="expT")
            nc.scalar.activation(expT[:, off:512 + W], sc[:, off:512 + W],
                                 Exp, scale=0.125)
            if k >= 0:
                # zero j > i in the leading 128-col diagonal block of each half
                for lo in (off, 512):
                    nc.gpsimd.affine_select(
                        out=expT[:, lo:lo + 128],
                        in_=expT[:, lo:lo + 128],
                        compare_op=mybir.AluOpType.is_ge,
                        fill=0.0, base=0,
                        pattern=[[1, 128]], channel_multiplier=-1,
                    )
            return expT, off, W

        def emit_avdn(bj, expT, off, W, aug, dn, first, last):
            # col-tiled AV pair: g_even -> psum partitions 0-63, g_odd -> 64-127
            nc.tensor.matmul(
                aug[0:DK, off:512], lhsT=vSv[:, bj, :], rhs=expT[:, off:512],
                start=first, stop=last, skip_group_check=True,
            )
            nc.tensor.matmul(
                aug[DK:128, off:512], lhsT=vSv[:, bj, :],
                rhs=expT[:, 512:512 + W],
                start=first, stop=last, skip_group_check=True,
            )
            # denominators: ones-column matmuls -> partitions 0 / 32 (concurrent)
            nc.tensor.matmul(
                dn[0:1, off:512], lhsT=onescol, rhs=expT[:, off:512],
                start=first, stop=last, skip_group_check=True,
            )
            nc.tensor.matmul(
                dn[32:33, off:512], lhsT=onescol, rhs=expT[:, 512:512 + W],
                start=first, stop=last, skip_group_check=True,
            )

        def emit_norm(ti, p, aug, dn):
            if debug_dumps and ti == 3 and p == 1:
                daug = const.tile([128, 512], f32, name="daug")
                nc.vector.tensor_copy(daug, aug)
                nc.sync.dma_start(out=dbg["d_aug"], in_=daug)
                ddn = const.tile([33, 512], f32, name="ddn")
                nc.vector.tensor_copy(ddn[0:1, :], dn[0:1, :])
                nc.vector.tensor_copy(ddn[32:33, :], dn[32:33, :])
                nc.sync.dma_start(out=dbg["d_dn"], in_=ddn)
            rcp = rcpp.tile([33, 512], f32, tag="rcp", name="rcp")
            rcpB = rcpp.tile([33, 512], bf16, tag="rcpB", name="rcpB")
            # full-tile op: reciprocal_approx_fast mis-executes on HW for APs
            # with base partition != 0 (probed), so cover rows 0..32 in one op
            # (rows 1-31 are junk-in/junk-out, initialized once below)
            nc.vector.reciprocal_approx_fast(rcp, dn[0:33, :])
            nc.vector.tensor_copy(rcpB, rcp)
            # broadcast recips over the pair's rows, reusing dn's bank
            nc.tensor.matmul(dn[0:DK, :], lhsT=onesB[0:1, :], rhs=rcpB[0:1, :],
                             start=True, stop=True, skip_group_check=True)
            nc.tensor.matmul(dn[DK:128, :], lhsT=onesB[32:33, :],
                             rhs=rcpB[32:33, :], start=True, stop=True,
                             skip_group_check=True)
            # DVE may read only one PSUM operand: stage bc in SBUF via ACT
            bcS = rcpp.tile([128, 512], bf16, tag="bcS", name="bcS")
            nc.scalar.copy(bcS, dn)
            if debug_dumps and ti == 3 and p == 1:
                dbcS = const.tile([128, 512], f32, name="dbcS")
                nc.vector.tensor_copy(dbcS, bcS)
                nc.sync.dma_start(out=dbg["d_bcS"], in_=dbcS)
            nc.vector.tensor_mul(outT[p][:, ti * 512:(ti + 1) * 512], aug, bcS)

        # ---- pipelined schedule ----
        fill = deque()

        def pump(n):
            for _ in range(n):
                if not fill:
                    return
                fill.popleft()()

        # warm the PE clock (HAM) with identity matmuls on resident SBUF
        # data while the input DMAs stream in
        warm = ps_qkv.tile([128, 512], f32, tag="ps_qkv", name="warm")
        for _ in range(28):
            nc.tensor.matmul(warm[:, 0:128], lhsT=ident_bf, rhs=ident_bf,
                             start=True, stop=True)
        # startup QKV(0) rotates through the (still idle) score banks so the
        # three mi-groups never stall on a single bank's evacuation
        for mi in range(3):
            for c in range(4):
                emit_qkv_chunk((0,), mi, c, pool=ps_sc, tg="ps_sc")
        emit_vt(0)

        # F fillers are queued first (popped one-per-strip, their psy-bank
        # evacuations hide under attention); QKV chunks last (dense, stall-
        # free end-of-tile drain). Half of F(1) shifts to ti=3 so its 32
        # strips stay covered.
        fgroups = {
            1: [(sb, mi2) for sb in range(0, 4) for mi2 in range(4)],
            2: [(sb, mi2) for sb in range(4, 8) for mi2 in range(4)][:8],
            3: [(sb, mi2) for sb in range(4, 8) for mi2 in range(4)][8:]
               + [(sb, mi2) for sb in range(8, 12) for mi2 in range(4)],
        }
        for ti in range(NI):
            for sb, mi2 in fgroups.get(ti, []):
                fill.append(lambda sb=sb, mi2=mi2: emit_f(sb, mi2))
            if ti < NI - 1:
                for mi in range(3):
                    for c in range(4):
                        fill.append(lambda ni=ti + 1, mi=mi, c=c:
                                    emit_qkv_chunk((ni,), mi, c))
                fill.append(lambda ni=ti + 1: emit_vt(ni))
            nstr = 4 * ti + 4
            for p in range(2):
                aug = ps_av.tile([128, 512], f32, tag="ps_av", name="aug")
                dn = ps_dn.tile([128, 512], f32, tag="ps_dn", name="dn")
                # keep rows 1-31 finite/nonzero and owned by this tile for the
                # full-tile reciprocal (row 0 is overwritten by the start=True
                # denominator matmul)
                nc.vector.memset(dn[0:32, :], 1.0)
                # full-width strip 0 first (uniform start=True write), then
                # diagonal strips early so their exp->mask->AV latency hides
                # under later strips' scores instead of the pair's tail
                order = [0] + list(range(nstr - 1, 0, -1))
                prev = None
                for idx, bj in enumerate(order):
                    cur = (bj, emit_scores(ti, p, bj))
                    pump(1)
                    if prev is not None:
                        emit_avdn(prev[0], *prev[1], aug, dn,
                                  first=(idx == 1), last=False)
                    prev = cur
                pump(1)
                emit_avdn(prev[0], *prev[1], aug, dn, first=False, last=True)
                emit_norm(ti, p, aug, dn)
            pump(len(fill))
        # tail: attention banks are idle now -- rotate the final F through the
        # freed ps_sc/av/dn banks with alternating evac engines so the PE
        # stays dense (and HAM stays warm) to the end
        tail_pools = [(ps_sc, "ps_sc"), (ps_av, "ps_av"),
                      (ps_sc, "ps_sc"), (ps_dn, "ps_dn")]
        for i, (sb, mi2) in enumerate(
                (sb, mi2) for sb in range(4 * (NI - 1), 4 * NI)
                for mi2 in range(4)):
            pool, tg = tail_pools[i % 4]
            psy = pool.tile([128, 512], f32, tag=tg, name="psyt")
            for ft in range(2):
                nc.tensor.matmul(
                    psy,
                    lhsT=outT[ft][:, sb * 128:(sb + 1) * 128],
                    rhs=woTv[:, ft, mi2 * 512:(mi2 + 1) * 512],
                    start=(ft == 0),
                    stop=(ft == 1),
                )
            ysb = ysbp.tile([128, 512], bf16, tag="ysbp", name="ysb")
            if i % 2 == 0:
                nc.vector.tensor_copy(ysb, psy)
            else:
                nc.scalar.copy(ysb, psy)
            nc.sync.dma_start(
                out=y_d[sb * 128:(sb + 1) * 128, mi2 * 512:(mi2 + 1) * 512],
                in_=ysb,
            )

        if debug_dumps:
            for nm, src in [("d_qkvT0", qkvT[0]), ("d_qkvT1", qkvT[1]),
                            ("d_qkvT2", qkvT[2]), ("d_kdup", kdup),
                            ("d_vS", vS), ("d_outT0", outT[0]),
                            ("d_outT1", outT[1])]:
                nc.sync.dma_start(out=dbg[nm], in_=src)

    nc.compile()
    return nc


def _get_nc():
    if "nc" not in _cache:
        _cache["nc"] = _build()
    return _cache["nc"]


def _in_maps(x, wq, bq, wk, bk, wv, bv, wo):
    import ml_dtypes

    bf = ml_dtypes.bfloat16
    x = np.asarray(x, np.float32)
    # xt[ni*128+p, dt*512+s'] = x[ni*512+s', dt*128+p]  (shared by all cores)
    xT = np.ascontiguousarray(x.T).astype(bf)                    # [d, s]
    xt = np.ascontiguousarray(
        xT.reshape(NT, 128, NI, 512).transpose(2, 1, 0, 3)
    ).reshape(NI * 128, NT * 512)
    maps = []
    for h in range(N_CORES):
        qs = slice(h * QF, (h + 1) * QF)
        ks = slice(h * DK, (h + 1) * DK)
        wqkv = np.concatenate([wq[qs], wk[ks], wv[ks]], axis=0)  # [384, 2048]
        wqkvT = np.ascontiguousarray(wqkv.T.astype(np.float32)).astype(bf)
        wt = np.ascontiguousarray(
            wqkvT.reshape(NT, 128, NF).transpose(1, 0, 2)
        ).reshape(128, NT * NF)
        woT = np.ascontiguousarray(wo[:, qs].T.astype(np.float32)).astype(bf)
        wob = np.ascontiguousarray(
            woT.reshape(2, 128, D).transpose(1, 0, 2)
        ).reshape(128, 2 * D)
        bqkv = np.concatenate([bq[qs], bk[ks], bv[ks]], axis=0).astype(np.float32)
        maps.append({
            "xt": xt,
            "wt": wt,
            "wob": wob,
            "bq": np.ascontiguousarray(bqkv[:, None]),
        })
    return maps


def _run(inputs, trace=False, tmpdir=None):
    from concourse.bass_utils import run_bass_kernel_spmd

    nc = _get_nc()
    x = np.asarray(inputs["x"])[0]
    maps = _in_maps(
        x,
        np.asarray(inputs["wq"]), np.asarray(inputs["bq"]),
        np.asarray(inputs["wk"]), np.asarray(inputs["bk"]),
        np.asarray(inputs["wv"]), np.asarray(inputs["bv"]),
        np.asarray(inputs["wo"]),
    )
    res = run_bass_kernel_spmd(
        nc, maps, list(range(N_CORES)), trace=trace, tmpdir=tmpdir
    )
    y = np.zeros((S, D), dtype=np.float32)
    for i in range(N_CORES):
        y += np.asarray(res.results[i]["y"]).astype(np.float32)
    y += np.asarray(inputs["bo"])[None, :]
    return y[None], res


def kernel(**inputs):
    y, _ = _run(inputs, trace=False)
    return y.astype(np.float32)


# revision 30
# speedup vs baseline: 1.0562x; 1.0021x over previous
"""GQA kernel for Trainium2, 8-way tensor-parallel over kv heads.

Problem (hardcoded): B=1, S=2048, D_MODEL=2048, HQ=32, HKV=8, DK=64, causal.
Sharding: core h owns kv head h and its 4 query heads. Weights are sliced,
transposed and cast to bf16 on host; x is replicated (transposed bf16). Each
core computes a partial y (its heads projected through its slice of wo); host
sums the 8 partials and adds bo.

On-chip dataflow per core (all matmuls bf16 with fp32 PSUM accumulation):
  xT, wqkvT, woT arrive pre-transposed -> no PE transposes for x/weights
  qkvT[f, s] projection, evacuated via DVE with fused per-partition bias
  scores: heads processed in pairs (g_even at array rows 0-63, g_odd at rows
    64-127 via duplicated K) -> the two 64-contraction matmuls run
    CONCURRENTLY in separate PE row groups (tile_position auto-derived)
  expT = exp(0.125 * scoresT) on ACT, causal strips only; diagonal blocks
    masked by gpsimd affine_select on the leading 128 columns only
  AV: col-tiled pair -- head g_even -> PSUM partitions 0-63, g_odd -> 64-127
    of one bank (concurrent), V is 64-wide; denominators via ones-column
    matmuls into partitions 0 / 32 of a second bank (concurrent pair)
  normalize: reciprocal_approx_fast + tiny broadcast matmuls + one DVE mul
  y = outT.T @ woT per 128-row block, interleaved into attention slack

Emission is software-pipelined with a filler queue: F(ti-1) and QKV(ti+1)
matmul chunks are popped between attention strips so PE stays busy while ACT
computes exp (ACT paces the attention phase).
"""

from collections import deque

import numpy as np

S = 2048
D = 2048
HQ, HKV, GRP, DK = 32, 8, 4, 64
QF = GRP * DK            # 256 query features per core
NF = QF + 2 * DK         # 384 projected features per core
N_CORES = 8
NT = S // 128            # 16 d-tiles
NI = S // 512            # 4 i-tiles

_cache = {}


def _build(debug_dumps=False):
    import concourse.bass as bass
    import concourse.mybir as mybir
    from concourse import bacc, tile
    from concourse.masks import make_identity
    from contextlib import ExitStack

    f32 = mybir.dt.float32
    bf16 = mybir.dt.bfloat16
    Exp = mybir.ActivationFunctionType.Exp

    nc = bacc.Bacc(
        "TRN2",
        target_bir_lowering=False,
        debug=False,
        enable_asserts=False,
        num_devices=N_CORES,
    )
    # host layouts (see _in_maps): xt[ni*128+p, dt*512+s'] = x[ni*512+s', dt*128+p]
    xt_d = nc.dram_tensor("xt", [NI * 128, NT * 512], bf16, kind="ExternalInput").ap()
    wt_d = nc.dram_tensor("wt", [128, NT * NF], bf16, kind="ExternalInput").ap()
    wo_d = nc.dram_tensor("wob", [128, 2 * D], bf16, kind="ExternalInput").ap()
    bq_d = nc.dram_tensor("bq", [NF, 1], f32, kind="ExternalInput").ap()
    y_d = nc.dram_tensor("y", [S, D], bf16, kind="ExternalOutput").ap()
    dbg = {}
    if debug_dumps:
        for nm, shp in [("d_qkvT0", [128, S]), ("d_qkvT1", [128, S]),
                        ("d_qkvT2", [128, S]), ("d_kdup", [128, S]),
                        ("d_vS", [128, NT * DK]), ("d_outT0", [128, S]),
                        ("d_outT1", [128, S])]:
            dbg[nm] = nc.dram_tensor(nm, shp, bf16, kind="ExternalOutput").ap()
        for nm, shp in [("d_aug", [128, 512]), ("d_dn", [33, 512]),
                        ("d_bcS", [128, 512])]:
            dbg[nm] = nc.dram_tensor(nm, shp, f32, kind="ExternalOutput").ap()

    with tile.TileContext(nc) as tc, ExitStack() as ctx:
        const = ctx.enter_context(tc.tile_pool(name="const", bufs=1))
        expp = ctx.enter_context(tc.tile_pool(name="expp", bufs=4))
        ysbp = ctx.enter_context(tc.tile_pool(name="ysbp", bufs=4))
        rcpp = ctx.enter_context(tc.tile_pool(name="rcpp", bufs=2))
        # PSUM (8 banks): sc 2x[128,1024]=4, av 1, dn/bc 1, qkv 1, f 1
        ps_sc = ctx.enter_context(tc.tile_pool(name="ps_sc", bufs=2, space="PSUM"))
        ps_av = ctx.enter_context(tc.tile_pool(name="ps_av", bufs=1, space="PSUM"))
        ps_dn = ctx.enter_context(tc.tile_pool(name="ps_dn", bufs=1, space="PSUM"))
        ps_qkv = ctx.enter_context(tc.tile_pool(name="ps_qkv", bufs=1, space="PSUM"))
        ps_f = ctx.enter_context(tc.tile_pool(name="ps_f", bufs=1, space="PSUM"))

        # ---- constants ----
        ident_bf = const.tile([128, 128], bf16)
        make_identity(nc, ident_bf)
        onescol = const.tile([128, 1], bf16)
        nc.gpsimd.memset(onescol, 1.0)
        onesB = const.tile([33, 64], bf16)
        nc.gpsimd.memset(onesB, 1.0)

        # ---- persistent SBUF ----
        XT = const.tile([128, NI * NT * 512], bf16)
        XTv = XT.rearrange("p (n t s) -> p n t s", n=NI, t=NT)
        wT = const.tile([128, NT * NF], bf16)
        wTv = wT.rearrange("p (t f) -> p t f", t=NT)
        woT = const.tile([128, 2 * D], bf16)
        woTv = woT.rearrange("p (t m) -> p t m", t=2)
        # qkvT[0] = Q heads g0|g1 (rows 0-63|64-127), [1] = g2|g3, [2] = K|V
        qkvT = [const.tile([128, S], bf16, name=f"qkvT{i}") for i in range(3)]
        kdup = const.tile([128, S], bf16)           # K duplicated at rows 64-127
        vS = const.tile([128, NT * DK], bf16)       # V as [s%128, strip, f]
        vSv = vS.rearrange("p (j f) -> p j f", j=NT)
        outT = [const.tile([128, S], bf16, name=f"outT{i}") for i in range(2)]
        btl = [const.tile([128, 1], f32, name=f"btl{i}") for i in range(3)]

        # ---- upfront DMAs (x slabs ordered by first use; the ni=0 slab and
        # wT arrive in dt-quad chunks so QKV(0)'s first matmuls start early) ----
        wTq = wT.rearrange("p (q r) -> p q r", q=4)
        for q in range(4):
            nc.sync.dma_start(out=XTv[:, 0, 4 * q:4 * q + 4, :],
                              in_=xt_d[0:128, q * 2048:(q + 1) * 2048])
            nc.sync.dma_start(out=wTq[:, q],
                              in_=wt_d[:, q * 4 * NF:(q + 1) * 4 * NF])
        for mi in range(3):
            nc.sync.dma_start(out=btl[mi], in_=bq_d[mi * 128:(mi + 1) * 128, :])
        nc.sync.dma_start(out=XTv[:, 1], in_=xt_d[128:256, :])
        nc.sync.dma_start(out=woT, in_=wo_d)
        nc.sync.dma_start(out=XTv[:, 2], in_=xt_d[256:384, :])
        nc.sync.dma_start(out=XTv[:, 3], in_=xt_d[384:512, :])

        # ---- stage emitters ----
        psq_live = {}

        def emit_qkv_chunk(nis, mi, c, pool=None, tg="ps_qkv"):
            # 4 of the 16 contraction tiles of the [128f, 512s] projection;
            # paired nis share the stationary weights (walrus dedupes the
            # LDWEIGHTS of consecutive same-lhsT matmuls)
            if c == 0:
                for ni in nis:
                    psq_live[(ni, mi)] = (pool or ps_qkv).tile(
                        [128, 512], f32, tag=tg, name="psq")
            for dt in range(4 * c, 4 * c + 4):
                for ni in nis:
                    nc.tensor.matmul(
                        psq_live[(ni, mi)],
                        lhsT=wTv[:, dt, mi * 128:(mi + 1) * 128],
                        rhs=XTv[:, ni, dt, :],
                        start=(dt == 0),
                        stop=(dt == NT - 1),
                    )
            if c == 3:
                for ni in nis:
                    psq = psq_live.pop((ni, mi))
                    nc.vector.tensor_scalar_add(
                        qkvT[mi][:, ni * 512:(ni + 1) * 512], psq, btl[mi])
                    if mi == 2:
                        nc.sync.dma_start(
                            out=kdup[64:128, ni * 512:(ni + 1) * 512],
                            in_=qkvT[2][0:DK, ni * 512:(ni + 1) * 512],
                        )

        def emit_vt(ni):
            # V strips of this i-tile transposed into vS via PE
            psv = ps_f.tile([128, 4 * DK], bf16, tag="ps_f", name="psv")
            for j in range(4):
                nc.tensor.transpose(
                    psv[:, j * DK:(j + 1) * DK],
                    qkvT[2][64:128, (4 * ni + j) * 128:(4 * ni + j + 1) * 128],
                    ident_bf[64:128, 64:128],
                )
            nc.vector.tensor_copy(
                vSv[:, 4 * ni:4 * ni + 4, :],
                psv.rearrange("p (a b) -> p a b", a=4),
            )

        def emit_f(sb, mi2):
            psy = ps_f.tile([128, 512], f32, tag="ps_f", name="psy")
            for ft in range(2):
                nc.tensor.matmul(
                    psy,
                    lhsT=outT[ft][:, sb * 128:(sb + 1) * 128],
                    rhs=woTv[:, ft, mi2 * 512:(mi2 + 1) * 512],
                    start=(ft == 0),
                    stop=(ft == 1),
                )
            ysb = ysbp.tile([128, 512], bf16, tag="ysbp", name="ysb")
            nc.vector.tensor_copy(ysb, psy)
            nc.sync.dma_start(
                out=y_d[sb * 128:(sb + 1) * 128, mi2 * 512:(mi2 + 1) * 512],
                in_=ysb,
            )

        def emit_scores(ti, p, bj):
            # head pair p: g_even at rows 0-63 (K source qkvT[2]), g_odd at
            # rows 64-127 (kdup). The two matmuls auto-derive tile_position
            # (0,0)/(64,0) -> they run concurrently in separate PE row groups.
            # Strips are computed full-width; causally dead columns are zeroed
            # after exp (keeps every PSUM byte initialized for the single exp).
            k = bj - 4 * ti
            off = 128 * k if k > 0 else 0
            W = 512 - off
            sc = ps_sc.tile([128, 1024], f32, tag="ps_sc", name="sc")
            # E half stored i-aligned at [off:512], O half packed at [512:512+W]
            # so the exp range [off:512+W] is contiguous and fully written
            nc.tensor.matmul(
                sc[:, off:512],
                lhsT=qkvT[2][0:DK, bj * 128:(bj + 1) * 128],
                rhs=qkvT[p][0:DK, ti * 512 + off:(ti + 1) * 512],
                start=True, stop=True,
            )
            nc.tensor.matmul(
                sc[:, 512:512 + W],
                lhsT=kdup[64:128, bj * 128:(bj + 1) * 128],
                rhs=qkvT[p][64:128, ti * 512 + off:(ti + 1) * 512],
                start=True, stop=True,
            )
            expT = expp.tile([128, 1024], bf16, tag="expp", name="expT")
            nc.scalar.activation(expT[:, off:512 + W], sc[:, off:512 + W],
                                 Exp, scale=0.125)
            if k >= 0:
                # zero j > i in the leading 128-col diagonal block of each half
                for lo in (off, 512):
                    nc.gpsimd.affine_select(
                        out=expT[:, lo:lo + 128],
                        in_=expT[:, lo:lo + 128],
                        compare_op=mybir.AluOpType.is_ge,
                        fill=0.0, base=0,
                        pattern=[[1, 128]], channel_multiplier=-1,
                    )
            return expT, off, W

        def emit_avdn(bj, expT, off, W, aug, dn, first, last):
            # col-tiled AV pair: g_even -> psum partitions 0-63, g_odd -> 64-127
            nc.tensor.matmul(
                aug[0:DK, off:512], lhsT=vSv[:, bj, :], rhs=expT[:, off:512],
                start=first, stop=last, skip_group_check=True,
            )
            nc.tensor.matmul(
                aug[DK:128, off:512], lhsT=vSv[:, bj, :],
                rhs=expT[:, 512:512 + W],
                start=first, stop=last, skip_group_check=True,
            )
            # denominators: ones-column matmuls -> partitions 0 / 32 (concurrent)
            nc.tensor.matmul(
                dn[0:1, off:512], lhsT=onescol, rhs=expT[:, off:512],
                start=first, stop=last, skip_group_check=True,
            )
            nc.tensor.matmul(
                dn[32:33, off:512], lhsT=onescol, rhs=expT[:, 512:512 + W],
                start=first, stop=last, skip_group_check=True,
            )

        def emit_norm_pre(ti, p, aug, dn):
            # DVE-only prefix of the normalization: runs while the PE moves on
            if debug_dumps and ti == 3 and p == 1:
                daug = const.tile([128, 512], f32, name="daug")
                nc.vector.tensor_copy(daug, aug)
                nc.sync.dma_start(out=dbg["d_aug"], in_=daug)
                ddn = const.tile([33, 512], f32, name="ddn")
                nc.vector.tensor_copy(ddn[0:1, :], dn[0:1, :])
                nc.vector.tensor_copy(ddn[32:33, :], dn[32:33, :])
                nc.sync.dma_start(out=dbg["d_dn"], in_=ddn)
            rcp = rcpp.tile([33, 512], f32, tag="rcp", name="rcp")
            rcpB = rcpp.tile([33, 512], bf16, tag="rcpB", name="rcpB")
            # full-tile op: reciprocal_approx_fast mis-executes on HW for APs
            # with base partition != 0 (probed), so cover rows 0..32 in one op
            # (rows 1-31 are junk-in/junk-out, initialized once below)
            nc.vector.reciprocal_approx_fast(rcp, dn[0:33, :])
            nc.vector.tensor_copy(rcpB, rcp)
            return rcpB

        def emit_norm_fin(ti, p, aug, dn, rcpB):
            # deferred finish: by flush time the reciprocals are ready, so the
            # broadcast matmuls no longer stall the PE queue head
            # broadcast recips over the pair's rows, reusing dn's bank
            nc.tensor.matmul(dn[0:DK, :], lhsT=onesB[0:1, :], rhs=rcpB[0:1, :],
                             start=True, stop=True, skip_group_check=True)
            nc.tensor.matmul(dn[DK:128, :], lhsT=onesB[32:33, :],
                             rhs=rcpB[32:33, :], start=True, stop=True,
                             skip_group_check=True)
            # DVE may read only one PSUM operand: stage bc in SBUF via ACT
            bcS = rcpp.tile([128, 512], bf16, tag="bcS", name="bcS")
            nc.scalar.copy(bcS, dn)
            if debug_dumps and ti == 3 and p == 1:
                dbcS = const.tile([128, 512], f32, name="dbcS")
                nc.vector.tensor_copy(dbcS, bcS)
                nc.sync.dma_start(out=dbg["d_bcS"], in_=dbcS)
            nc.vector.tensor_mul(outT[p][:, ti * 512:(ti + 1) * 512], aug, bcS)

        # ---- pipelined schedule ----
        fill = deque()

        def pump(n):
            for _ in range(n):
                if not fill:
                    return
                fill.popleft()()

        # warm the PE clock (HAM) with identity matmuls on resident SBUF
        # data while the input DMAs stream in
        warm = ps_qkv.tile([128, 512], f32, tag="ps_qkv", name="warm")
        for _ in range(28):
            nc.tensor.matmul(warm[:, 0:128], lhsT=ident_bf, rhs=ident_bf,
                             start=True, stop=True)
        # startup QKV(0) rotates through the (still idle) score banks so the
        # three mi-groups never stall on a single bank's evacuation
        for mi in range(3):
            for c in range(4):
                emit_qkv_chunk((0,), mi, c, pool=ps_sc, tg="ps_sc")
        emit_vt(0)

        # F fillers are queued first (popped one-per-strip, their psy-bank
        # evacuations hide under attention); QKV chunks last (dense, stall-
        # free end-of-tile drain). Half of F(1) shifts to ti=3 so its 32
        # strips stay covered.
        pending_norm = [None]
        fgroups = {
            1: [(sb, mi2) for sb in range(0, 4) for mi2 in range(4)],
            2: [(sb, mi2) for sb in range(4, 8) for mi2 in range(4)][:8],
            3: [(sb, mi2) for sb in range(4, 8) for mi2 in range(4)][8:]
               + [(sb, mi2) for sb in range(8, 12) for mi2 in range(4)],
        }
        for ti in range(NI):
            for sb, mi2 in fgroups.get(ti, []):
                fill.append(lambda sb=sb, mi2=mi2: emit_f(sb, mi2))
            if ti < NI - 1:
                for mi in range(3):
                    for c in range(4):
                        fill.append(lambda ni=ti + 1, mi=mi, c=c:
                                    emit_qkv_chunk((ni,), mi, c))
                fill.append(lambda ni=ti + 1: emit_vt(ni))
            nstr = 4 * ti + 4
            for p in range(2):
                # full-width strip 0 first (uniform start=True write), then
                # diagonal strips early so their exp->mask->AV latency hides
                # under later strips' scores instead of the pair's tail
                order = [0] + list(range(nstr - 1, 0, -1))
                prev = None
                aug = dn = None
                for idx, bj in enumerate(order):
                    cur = (bj, emit_scores(ti, p, bj))
                    if idx == 0:
                        # flush the previous pair's deferred norm finish, then
                        # allocate this pair's banks (order matters: the old
                        # tiles' last ops must be emitted before recycling)
                        if pending_norm[0] is not None:
                            emit_norm_fin(*pending_norm[0])
                            pending_norm[0] = None
                        aug = ps_av.tile([128, 512], f32, tag="ps_av",
                                         name="aug")
                        dn = ps_dn.tile([128, 512], f32, tag="ps_dn",
                                        name="dn")
                        # keep rows 1-31 finite/nonzero and owned by this tile
                        # for the full-tile reciprocal (row 0 is overwritten
                        # by the start=True denominator matmul)
                        nc.vector.memset(dn[0:32, :], 1.0)
                    pump(1)
                    if prev is not None:
                        emit_avdn(prev[0], *prev[1], aug, dn,
                                  first=(idx == 1), last=False)
                    prev = cur
                pump(1)
                emit_avdn(prev[0], *prev[1], aug, dn, first=False, last=True)
                rcpB = emit_norm_pre(ti, p, aug, dn)
                pending_norm[0] = (ti, p, aug, dn, rcpB)
            pump(len(fill))
        if pending_norm[0] is not None:
            emit_norm_fin(*pending_norm[0])
            pending_norm[0] = None
        # tail: attention banks are idle now -- rotate the final F through the
        # freed ps_sc/av/dn banks with alternating evac engines so the PE
        # stays dense (and HAM stays warm) to the end
        tail_pools = [(ps_sc, "ps_sc"), (ps_av, "ps_av"),
                      (ps_sc, "ps_sc"), (ps_dn, "ps_dn")]
        for i, (sb, mi2) in enumerate(
                (sb, mi2) for sb in range(4 * (NI - 1), 4 * NI)
                for mi2 in range(4)):
            pool, tg = tail_pools[i % 4]
            psy = pool.tile([128, 512], f32, tag=tg, name="psyt")
            for ft in range(2):
                nc.tensor.matmul(
                    psy,
                    lhsT=outT[ft][:, sb * 128:(sb + 1) * 128],
                    rhs=woTv[:, ft, mi2 * 512:(mi2 + 1) * 512],
                    start=(ft == 0),
                    stop=(ft == 1),
                )
            ysb = ysbp.tile([128, 512], bf16, tag="ysbp", name="ysb")
            if i % 2 == 0:
                nc.vector.tensor_copy(ysb, psy)
            else:
                nc.scalar.copy(ysb, psy)
            nc.sync.dma_start(
                out=y_d[sb * 128:(sb + 1) * 128, mi2 * 512:(mi2 + 1) * 512],
                in_=ysb,
            )

        if debug_dumps:
            for nm, src in [("d_qkvT0", qkvT[0]), ("d_qkvT1", qkvT[1]),
                            ("d_qkvT2", qkvT[2]), ("d_kdup", kdup),
                            ("d_vS", vS), ("d_outT0", outT[0]),
                            ("d_outT1", outT[1])]:
                nc.sync.dma_start(out=dbg[nm], in_=src)

    nc.compile()
    return nc


def _get_nc():
    if "nc" not in _cache:
        _cache["nc"] = _build()
    return _cache["nc"]


def _in_maps(x, wq, bq, wk, bk, wv, bv, wo):
    import ml_dtypes

    bf = ml_dtypes.bfloat16
    x = np.asarray(x, np.float32)
    # xt[ni*128+p, dt*512+s'] = x[ni*512+s', dt*128+p]  (shared by all cores)
    xT = np.ascontiguousarray(x.T).astype(bf)                    # [d, s]
    xt = np.ascontiguousarray(
        xT.reshape(NT, 128, NI, 512).transpose(2, 1, 0, 3)
    ).reshape(NI * 128, NT * 512)
    maps = []
    for h in range(N_CORES):
        qs = slice(h * QF, (h + 1) * QF)
        ks = slice(h * DK, (h + 1) * DK)
        wqkv = np.concatenate([wq[qs], wk[ks], wv[ks]], axis=0)  # [384, 2048]
        wqkvT = np.ascontiguousarray(wqkv.T.astype(np.float32)).astype(bf)
        wt = np.ascontiguousarray(
            wqkvT.reshape(NT, 128, NF).transpose(1, 0, 2)
        ).reshape(128, NT * NF)
        woT = np.ascontiguousarray(wo[:, qs].T.astype(np.float32)).astype(bf)
        wob = np.ascontiguousarray(
            woT.reshape(2, 128, D).transpose(1, 0, 2)
        ).reshape(128, 2 * D)
        bqkv = np.concatenate([bq[qs], bk[ks], bv[ks]], axis=0).astype(np.float32)
        maps.append({
            "xt": xt,
            "wt": wt,
            "wob": wob,
            "bq": np.ascontiguousarray(bqkv[:, None]),
        })
    return maps


def _run(inputs, trace=False, tmpdir=None):
    from concourse.bass_utils import run_bass_kernel_spmd

    nc = _get_nc()
    x = np.asarray(inputs["x"])[0]
    maps = _in_maps(
        x,
        np.asarray(inputs["wq"]), np.asarray(inputs["bq"]),
        np.asarray(inputs["wk"]), np.asarray(inputs["bk"]),
        np.asarray(inputs["wv"]), np.asarray(inputs["bv"]),
        np.asarray(inputs["wo"]),
    )
    res = run_bass_kernel_spmd(
        nc, maps, list(range(N_CORES)), trace=trace, tmpdir=tmpdir
    )
    y = np.zeros((S, D), dtype=np.float32)
    for i in range(N_CORES):
        y += np.asarray(res.results[i]["y"]).astype(np.float32)
    y += np.asarray(inputs["bo"])[None, :]
    return y[None], res


def kernel(**inputs):
    y, _ = _run(inputs, trace=False)
    return y.astype(np.float32)


# revision 31
# speedup vs baseline: 1.0681x; 1.0112x over previous
"""GQA kernel for Trainium2, 8-way tensor-parallel over kv heads.

Problem (hardcoded): B=1, S=2048, D_MODEL=2048, HQ=32, HKV=8, DK=64, causal.
Sharding: core h owns kv head h and its 4 query heads. Weights are sliced,
transposed and cast to bf16 on host; x is replicated (transposed bf16). Each
core computes a partial y (its heads projected through its slice of wo); host
sums the 8 partials and adds bo.

On-chip dataflow per core (all matmuls bf16 with fp32 PSUM accumulation):
  xT, wqkvT, woT arrive pre-transposed -> no PE transposes for x/weights
  qkvT[f, s] projection, evacuated via DVE with fused per-partition bias
  scores: heads processed in pairs (g_even at array rows 0-63, g_odd at rows
    64-127 via duplicated K) -> the two 64-contraction matmuls run
    CONCURRENTLY in separate PE row groups (tile_position auto-derived)
  expT = exp(0.125 * scoresT) on ACT, causal strips only; diagonal blocks
    masked by gpsimd affine_select on the leading 128 columns only
  AV: col-tiled pair -- head g_even -> PSUM partitions 0-63, g_odd -> 64-127
    of one bank (concurrent), V is 64-wide; denominators via ones-column
    matmuls into partitions 0 / 32 of a second bank (concurrent pair)
  normalize: reciprocal_approx_fast + tiny broadcast matmuls + one DVE mul
  y = outT.T @ woT per 128-row block, interleaved into attention slack

Emission is software-pipelined with a filler queue: F(ti-1) and QKV(ti+1)
matmul chunks are popped between attention strips so PE stays busy while ACT
computes exp (ACT paces the attention phase).
"""

from collections import deque

import numpy as np

S = 2048
D = 2048
HQ, HKV, GRP, DK = 32, 8, 4, 64
QF = GRP * DK            # 256 query features per core
NF = QF + 2 * DK         # 384 projected features per core
N_CORES = 8
NT = S // 128            # 16 d-tiles
NI = S // 512            # 4 i-tiles

_cache = {}


def _build(debug_dumps=False):
    import concourse.bass as bass
    import concourse.mybir as mybir
    from concourse import bacc, tile
    from concourse.masks import make_identity
    from contextlib import ExitStack

    f32 = mybir.dt.float32
    bf16 = mybir.dt.bfloat16
    Exp = mybir.ActivationFunctionType.Exp

    nc = bacc.Bacc(
        "TRN2",
        target_bir_lowering=False,
        debug=False,
        enable_asserts=False,
        num_devices=N_CORES,
    )
    # host layouts (see _in_maps): xt[ni*128+p, dt*512+s'] = x[ni*512+s', dt*128+p]
    xt_d = nc.dram_tensor("xt", [NI * 128, NT * 512], bf16, kind="ExternalInput").ap()
    wt_d = nc.dram_tensor("wt", [128, NT * NF], bf16, kind="ExternalInput").ap()
    wo_d = nc.dram_tensor("wob", [128, 2 * D], bf16, kind="ExternalInput").ap()
    bq_d = nc.dram_tensor("bq", [NF, 1], f32, kind="ExternalInput").ap()
    y_d = nc.dram_tensor("y", [S, D], bf16, kind="ExternalOutput").ap()
    dbg = {}
    if debug_dumps:
        for nm, shp in [("d_qkvT0", [128, S]), ("d_qkvT1", [128, S]),
                        ("d_qkvT2", [128, S]), ("d_kdup", [128, S]),
                        ("d_vS", [128, NT * DK]), ("d_outT0", [128, S]),
                        ("d_outT1", [128, S])]:
            dbg[nm] = nc.dram_tensor(nm, shp, bf16, kind="ExternalOutput").ap()
        for nm, shp in [("d_aug", [128, 512]), ("d_dn", [33, 512]),
                        ("d_bcS", [128, 512])]:
            dbg[nm] = nc.dram_tensor(nm, shp, f32, kind="ExternalOutput").ap()

    with tile.TileContext(nc) as tc, ExitStack() as ctx:
        const = ctx.enter_context(tc.tile_pool(name="const", bufs=1))
        expp = ctx.enter_context(tc.tile_pool(name="expp", bufs=4))
        ysbp = ctx.enter_context(tc.tile_pool(name="ysbp", bufs=4))
        rcpp = ctx.enter_context(tc.tile_pool(name="rcpp", bufs=2))
        # PSUM (8 banks): sc 2x[128,1024]=4, av 1, dn/bc 1, qkv 1, f 1
        ps_sc = ctx.enter_context(tc.tile_pool(name="ps_sc", bufs=2, space="PSUM"))
        ps_av = ctx.enter_context(tc.tile_pool(name="ps_av", bufs=1, space="PSUM"))
        ps_dn = ctx.enter_context(tc.tile_pool(name="ps_dn", bufs=1, space="PSUM"))
        ps_qkv = ctx.enter_context(tc.tile_pool(name="ps_qkv", bufs=1, space="PSUM"))
        ps_f = ctx.enter_context(tc.tile_pool(name="ps_f", bufs=1, space="PSUM"))

        # ---- constants ----
        ident_bf = const.tile([128, 128], bf16)
        make_identity(nc, ident_bf)
        onescol = const.tile([128, 1], bf16)
        nc.gpsimd.memset(onescol, 1.0)
        onesB = const.tile([33, 64], bf16)
        nc.gpsimd.memset(onesB, 1.0)

        # ---- persistent SBUF ----
        XT = const.tile([128, NI * NT * 512], bf16)
        XTv = XT.rearrange("p (n t s) -> p n t s", n=NI, t=NT)
        wT = const.tile([128, NT * NF], bf16)
        wTv = wT.rearrange("p (t f) -> p t f", t=NT)
        woT = const.tile([128, 2 * D], bf16)
        woTv = woT.rearrange("p (t m) -> p t m", t=2)
        # qkvT[0] = Q heads g0|g1 (rows 0-63|64-127), [1] = g2|g3, [2] = K|V
        qkvT = [const.tile([128, S], bf16, name=f"qkvT{i}") for i in range(3)]
        kdup = const.tile([128, S], bf16)           # K duplicated at rows 64-127
        vS = const.tile([128, NT * DK], bf16)       # V as [s%128, strip, f]
        vSv = vS.rearrange("p (j f) -> p j f", j=NT)
        outT = [const.tile([128, S], bf16, name=f"outT{i}") for i in range(2)]
        btl = [const.tile([128, 1], f32, name=f"btl{i}") for i in range(3)]

        # ---- upfront DMAs (x slabs ordered by first use; the ni=0 slab and
        # wT arrive in dt-quad chunks so QKV(0)'s first matmuls start early) ----
        wTq = wT.rearrange("p (q r) -> p q r", q=4)
        for q in range(4):
            nc.sync.dma_start(out=XTv[:, 0, 4 * q:4 * q + 4, :],
                              in_=xt_d[0:128, q * 2048:(q + 1) * 2048])
            nc.sync.dma_start(out=wTq[:, q],
                              in_=wt_d[:, q * 4 * NF:(q + 1) * 4 * NF])
        for mi in range(3):
            nc.sync.dma_start(out=btl[mi], in_=bq_d[mi * 128:(mi + 1) * 128, :])
        nc.sync.dma_start(out=XTv[:, 1], in_=xt_d[128:256, :])
        nc.sync.dma_start(out=woT, in_=wo_d)
        nc.sync.dma_start(out=XTv[:, 2], in_=xt_d[256:384, :])
        nc.sync.dma_start(out=XTv[:, 3], in_=xt_d[384:512, :])

        # ---- stage emitters ----
        psq_live = {}

        def emit_qkv_chunk(nis, mi, c, pool=None, tg="ps_qkv"):
            # 4 of the 16 contraction tiles of the [128f, 512s] projection;
            # paired nis share the stationary weights (walrus dedupes the
            # LDWEIGHTS of consecutive same-lhsT matmuls)
            if c == 0:
                for ni in nis:
                    psq_live[(ni, mi)] = (pool or ps_qkv).tile(
                        [128, 512], f32, tag=tg, name="psq")
            for dt in range(4 * c, 4 * c + 4):
                for ni in nis:
                    nc.tensor.matmul(
                        psq_live[(ni, mi)],
                        lhsT=wTv[:, dt, mi * 128:(mi + 1) * 128],
                        rhs=XTv[:, ni, dt, :],
                        start=(dt == 0),
                        stop=(dt == NT - 1),
                    )
            if c == 3:
                for ni in nis:
                    psq = psq_live.pop((ni, mi))
                    nc.vector.tensor_scalar_add(
                        qkvT[mi][:, ni * 512:(ni + 1) * 512], psq, btl[mi])
                    if mi == 2:
                        nc.sync.dma_start(
                            out=kdup[64:128, ni * 512:(ni + 1) * 512],
                            in_=qkvT[2][0:DK, ni * 512:(ni + 1) * 512],
                        )

        def emit_vt(ni):
            # V strips of this i-tile transposed into vS via PE
            psv = ps_f.tile([128, 4 * DK], bf16, tag="ps_f", name="psv")
            for j in range(4):
                nc.tensor.transpose(
                    psv[:, j * DK:(j + 1) * DK],
                    qkvT[2][64:128, (4 * ni + j) * 128:(4 * ni + j + 1) * 128],
                    ident_bf[64:128, 64:128],
                )
            nc.vector.tensor_copy(
                vSv[:, 4 * ni:4 * ni + 4, :],
                psv.rearrange("p (a b) -> p a b", a=4),
            )

        def emit_f(sb, mi2):
            psy = ps_f.tile([128, 512], f32, tag="ps_f", name="psy")
            for ft in range(2):
                nc.tensor.matmul(
                    psy,
                    lhsT=outT[ft][:, sb * 128:(sb + 1) * 128],
                    rhs=woTv[:, ft, mi2 * 512:(mi2 + 1) * 512],
                    start=(ft == 0),
                    stop=(ft == 1),
                )
            ysb = ysbp.tile([128, 512], bf16, tag="ysbp", name="ysb")
            nc.vector.tensor_copy(ysb, psy)
            nc.sync.dma_start(
                out=y_d[sb * 128:(sb + 1) * 128, mi2 * 512:(mi2 + 1) * 512],
                in_=ysb,
            )

        def emit_scores(ti, p, bj):
            # head pair p: g_even at rows 0-63 (K source qkvT[2]), g_odd at
            # rows 64-127 (kdup). The two matmuls auto-derive tile_position
            # (0,0)/(64,0) -> they run concurrently in separate PE row groups.
            # Strips are computed full-width; causally dead columns are zeroed
            # after exp (keeps every PSUM byte initialized for the single exp).
            k = bj - 4 * ti
            off = 128 * k if k > 0 else 0
            W = 512 - off
            sc = ps_sc.tile([128, 1024], f32, tag="ps_sc", name="sc")
            # E half stored i-aligned at [off:512], O half packed at [512:512+W]
            # so the exp range [off:512+W] is contiguous and fully written
            nc.tensor.matmul(
                sc[:, off:512],
                lhsT=qkvT[2][0:DK, bj * 128:(bj + 1) * 128],
                rhs=qkvT[p][0:DK, ti * 512 + off:(ti + 1) * 512],
                start=True, stop=True,
            )
            nc.tensor.matmul(
                sc[:, 512:512 + W],
                lhsT=kdup[64:128, bj * 128:(bj + 1) * 128],
                rhs=qkvT[p][64:128, ti * 512 + off:(ti + 1) * 512],
                start=True, stop=True,
            )
            expT = expp.tile([128, 1024], bf16, tag="expp", name="expT")
            nc.scalar.activation(expT[:, off:512 + W], sc[:, off:512 + W],
                                 Exp, scale=0.125)
            if k >= 0:
                # zero j > i in the leading 128-col diagonal block of each half
                for lo in (off, 512):
                    nc.gpsimd.affine_select(
                        out=expT[:, lo:lo + 128],
                        in_=expT[:, lo:lo + 128],
                        compare_op=mybir.AluOpType.is_ge,
                        fill=0.0, base=0,
                        pattern=[[1, 128]], channel_multiplier=-1,
                    )
            return expT, off, W

        def emit_avdn(bj, expT, off, W, aug, dn, first, last):
            # col-tiled AV pair: g_even -> psum partitions 0-63, g_odd -> 64-127
            nc.tensor.matmul(
                aug[0:DK, off:512], lhsT=vSv[:, bj, :], rhs=expT[:, off:512],
                start=first, stop=last, skip_group_check=True,
            )
            nc.tensor.matmul(
                aug[DK:128, off:512], lhsT=vSv[:, bj, :],
                rhs=expT[:, 512:512 + W],
                start=first, stop=last, skip_group_check=True,
            )
            # denominators: ones-column matmuls -> partitions 0 / 32 (concurrent)
            nc.tensor.matmul(
                dn[0:1, off:512], lhsT=onescol, rhs=expT[:, off:512],
                start=first, stop=last, skip_group_check=True,
            )
            nc.tensor.matmul(
                dn[32:33, off:512], lhsT=onescol, rhs=expT[:, 512:512 + W],
                start=first, stop=last, skip_group_check=True,
            )

        def emit_norm_pre(ti, p, aug, dn):
            # DVE-only prefix of the normalization: runs while the PE moves on
            if debug_dumps and ti == 3 and p == 1:
                daug = const.tile([128, 512], f32, name="daug")
                nc.vector.tensor_copy(daug, aug)
                nc.sync.dma_start(out=dbg["d_aug"], in_=daug)
                ddn = const.tile([33, 512], f32, name="ddn")
                nc.vector.tensor_copy(ddn[0:1, :], dn[0:1, :])
                nc.vector.tensor_copy(ddn[32:33, :], dn[32:33, :])
                nc.sync.dma_start(out=dbg["d_dn"], in_=ddn)
            rcp = rcpp.tile([33, 512], f32, tag="rcp", name="rcp")
            rcpB = rcpp.tile([33, 512], bf16, tag="rcpB", name="rcpB")
            # full-tile op: reciprocal_approx_fast mis-executes on HW for APs
            # with base partition != 0 (probed), so cover rows 0..32 in one op
            # (rows 1-31 are junk-in/junk-out, initialized once below)
            nc.vector.reciprocal_approx_fast(rcp, dn[0:33, :])
            nc.vector.tensor_copy(rcpB, rcp)
            return rcpB

        def emit_norm_fin(ti, p, aug, dn, rcpB):
            # deferred finish: by flush time the reciprocals are ready, so the
            # broadcast matmuls no longer stall the PE queue head
            # broadcast recips over the pair's rows, reusing dn's bank
            nc.tensor.matmul(dn[0:DK, :], lhsT=onesB[0:1, :], rhs=rcpB[0:1, :],
                             start=True, stop=True, skip_group_check=True)
            nc.tensor.matmul(dn[DK:128, :], lhsT=onesB[32:33, :],
                             rhs=rcpB[32:33, :], start=True, stop=True,
                             skip_group_check=True)
            # DVE may read only one PSUM operand: stage bc in SBUF via ACT
            bcS = rcpp.tile([128, 512], bf16, tag="bcS", name="bcS")
            nc.scalar.copy(bcS, dn)
            if debug_dumps and ti == 3 and p == 1:
                dbcS = const.tile([128, 512], f32, name="dbcS")
                nc.vector.tensor_copy(dbcS, bcS)
                nc.sync.dma_start(out=dbg["d_bcS"], in_=dbcS)
            nc.vector.tensor_mul(outT[p][:, ti * 512:(ti + 1) * 512], aug, bcS)

        # ---- pipelined schedule ----
        fill = deque()

        def pump(n):
            for _ in range(n):
                if not fill:
                    return
                fill.popleft()()

        # warm the PE clock (HAM) with identity matmuls on resident SBUF
        # data while the input DMAs stream in
        warm = ps_qkv.tile([128, 512], f32, tag="ps_qkv", name="warm")
        for _ in range(48):
            nc.tensor.matmul(warm[:, 0:128], lhsT=ident_bf, rhs=ident_bf,
                             start=True, stop=True)
        # startup QKV(0) rotates through the (still idle) score banks so the
        # three mi-groups never stall on a single bank's evacuation
        for mi in range(3):
            for c in range(4):
                emit_qkv_chunk((0,), mi, c, pool=ps_sc, tg="ps_sc")
        emit_vt(0)

        # F fillers are queued first (popped one-per-strip, their psy-bank
        # evacuations hide under attention); QKV chunks last (dense, stall-
        # free end-of-tile drain). Half of F(1) shifts to ti=3 so its 32
        # strips stay covered.
        pending_norm = [None]
        fgroups = {
            1: [(sb, mi2) for sb in range(0, 4) for mi2 in range(4)],
            2: [(sb, mi2) for sb in range(4, 8) for mi2 in range(4)][:8],
            3: [(sb, mi2) for sb in range(4, 8) for mi2 in range(4)][8:]
               + [(sb, mi2) for sb in range(8, 12) for mi2 in range(4)],
        }
        for ti in range(NI):
            for sb, mi2 in fgroups.get(ti, []):
                fill.append(lambda sb=sb, mi2=mi2: emit_f(sb, mi2))
            if ti < NI - 1:
                for mi in range(3):
                    for c in range(4):
                        fill.append(lambda ni=ti + 1, mi=mi, c=c:
                                    emit_qkv_chunk(
                                        (ni,), mi, c,
                                        pool=ps_f if mi == 1 else None,
                                        tg="ps_f" if mi == 1 else "ps_qkv"))
                fill.append(lambda ni=ti + 1: emit_vt(ni))
            nstr = 4 * ti + 4
            for p in range(2):
                # full-width strip 0 first (uniform start=True write), then
                # diagonal strips early so their exp->mask->AV latency hides
                # under later strips' scores instead of the pair's tail
                order = [0] + list(range(nstr - 1, 0, -1))
                prev = None
                aug = dn = None
                for idx, bj in enumerate(order):
                    cur = (bj, emit_scores(ti, p, bj))
                    if idx == 0:
                        # flush the previous pair's deferred norm finish, then
                        # allocate this pair's banks (order matters: the old
                        # tiles' last ops must be emitted before recycling)
                        if pending_norm[0] is not None:
                            emit_norm_fin(*pending_norm[0])
                            pending_norm[0] = None
                        aug = ps_av.tile([128, 512], f32, tag="ps_av",
                                         name="aug")
                        dn = ps_dn.tile([128, 512], f32, tag="ps_dn",
                                        name="dn")
                        # keep rows 1-31 finite/nonzero and owned by this tile
                        # for the full-tile reciprocal (row 0 is overwritten
                        # by the start=True denominator matmul)
                        nc.vector.memset(dn[0:32, :], 1.0)
                    pump(1)
                    if prev is not None:
                        emit_avdn(prev[0], *prev[1], aug, dn,
                                  first=(idx == 1), last=False)
                    prev = cur
                pump(1)
                emit_avdn(prev[0], *prev[1], aug, dn, first=False, last=True)
                rcpB = emit_norm_pre(ti, p, aug, dn)
                pending_norm[0] = (ti, p, aug, dn, rcpB)
            pump(len(fill))
        if pending_norm[0] is not None:
            emit_norm_fin(*pending_norm[0])
            pending_norm[0] = None
        # tail: attention banks are idle now -- rotate the final F through the
        # freed ps_sc/av/dn banks with alternating evac engines so the PE
        # stays dense (and HAM stays warm) to the end
        tail_pools = [(ps_sc, "ps_sc"), (ps_av, "ps_av"),
                      (ps_sc, "ps_sc"), (ps_dn, "ps_dn")]
        for i, (sb, mi2) in enumerate(
                (sb, mi2) for sb in range(4 * (NI - 1), 4 * NI)
                for mi2 in range(4)):
            pool, tg = tail_pools[i % 4]
            psy = pool.tile([128, 512], f32, tag=tg, name="psyt")
            for ft in range(2):
                nc.tensor.matmul(
                    psy,
                    lhsT=outT[ft][:, sb * 128:(sb + 1) * 128],
                    rhs=woTv[:, ft, mi2 * 512:(mi2 + 1) * 512],
                    start=(ft == 0),
                    stop=(ft == 1),
                )
            ysb = ysbp.tile([128, 512], bf16, tag="ysbp", name="ysb")
            if i % 2 == 0:
                nc.vector.tensor_copy(ysb, psy)
            else:
                nc.scalar.copy(ysb, psy)
            nc.sync.dma_start(
                out=y_d[sb * 128:(sb + 1) * 128, mi2 * 512:(mi2 + 1) * 512],
                in_=ysb,
            )

        if debug_dumps:
            for nm, src in [("d_qkvT0", qkvT[0]), ("d_qkvT1", qkvT[1]),
                            ("d_qkvT2", qkvT[2]), ("d_kdup", kdup),
                            ("d_vS", vS), ("d_outT0", outT[0]),
                            ("d_outT1", outT[1])]:
                nc.sync.dma_start(out=dbg[nm], in_=src)

    nc.compile()
    return nc


def _get_nc():
    if "nc" not in _cache:
        _cache["nc"] = _build()
    return _cache["nc"]


def _in_maps(x, wq, bq, wk, bk, wv, bv, wo):
    import ml_dtypes

    bf = ml_dtypes.bfloat16
    x = np.asarray(x, np.float32)
    # xt[ni*128+p, dt*512+s'] = x[ni*512+s', dt*128+p]  (shared by all cores)
    xT = np.ascontiguousarray(x.T).astype(bf)                    # [d, s]
    xt = np.ascontiguousarray(
        xT.reshape(NT, 128, NI, 512).transpose(2, 1, 0, 3)
    ).reshape(NI * 128, NT * 512)
    maps = []
    for h in range(N_CORES):
        qs = slice(h * QF, (h + 1) * QF)
        ks = slice(h * DK, (h + 1) * DK)
        wqkv = np.concatenate([wq[qs], wk[ks], wv[ks]], axis=0)  # [384, 2048]
        wqkvT = np.ascontiguousarray(wqkv.T.astype(np.float32)).astype(bf)
        wt = np.ascontiguousarray(
            wqkvT.reshape(NT, 128, NF).transpose(1, 0, 2)
        ).reshape(128, NT * NF)
        woT = np.ascontiguousarray(wo[:, qs].T.astype(np.float32)).astype(bf)
        wob = np.ascontiguousarray(
            woT.reshape(2, 128, D).transpose(1, 0, 2)
        ).reshape(128, 2 * D)
        bqkv = np.concatenate([bq[qs], bk[ks], bv[ks]], axis=0).astype(np.float32)
        maps.append({
            "xt": xt,
            "wt": wt,
            "wob": wob,
            "bq": np.ascontiguousarray(bqkv[:, None]),
        })
    return maps


def _run(inputs, trace=False, tmpdir=None):
    from concourse.bass_utils import run_bass_kernel_spmd

    nc = _get_nc()
    x = np.asarray(inputs["x"])[0]
    maps = _in_maps(
        x,
        np.asarray(inputs["wq"]), np.asarray(inputs["bq"]),
        np.asarray(inputs["wk"]), np.asarray(inputs["bk"]),
        np.asarray(inputs["wv"]), np.asarray(inputs["bv"]),
        np.asarray(inputs["wo"]),
    )
    res = run_bass_kernel_spmd(
        nc, maps, list(range(N_CORES)), trace=trace, tmpdir=tmpdir
    )
    y = np.zeros((S, D), dtype=np.float32)
    for i in range(N_CORES):
        y += np.asarray(res.results[i]["y"]).astype(np.float32)
    y += np.asarray(inputs["bo"])[None, :]
    return y[None], res


def kernel(**inputs):
    y, _ = _run(inputs, trace=False)
    return y.astype(np.float32)
